# revision 12
# baseline (speedup 1.0000x reference)
"""Trainium2 Bass kernel for nn_Encoder (conv stack + VQ codebook).

Reference computation (fp32):
  x = mels [32, 80, 1024]
  5x (conv1d + batchnorm-affine + relu), 1x 1x1-conv + bias  -> z [32, 64, 511]
  VQ: nearest codebook row (squared L2, 512 codes, D=64) -> q_st, loss, perplexity

Sharding: data-parallel over batch across 8 NeuronCores (4 batches/core);
conv weights + codebook replicated.

Speed tricks (all verified to keep VQ argmins identical to the fp32 ref):
- fp16 hi/lo 3-term matmuls (W=Wh+Wl, X=Xh+Xl; Y ~= Wh.Xh + Wh.Xl + Wl.Xh):
  1 PE cycle/row instead of fp32's 4, output rel err ~3e-6 (the PE supports
  fp16 subnormals, so the lo parts need no scaling).
- Winograd F(2,3) on the stride-1 K=3 768x768 layers (L2/L4/L5): 4 transformed
  256-col matmuls per output pair instead of 3x511 direct columns (1.5x fewer
  PE cycles), transforms on DVE/ACT in fp32.
- The VQ distance computation stays fully fp32 and mirrors the reference's
  operation order; host finishes gather/loss/perplexity exactly as the
  reference does.
"""

import numpy as np

import concourse.bass as bass
import concourse.tile as tile
from concourse import bacc, mybir
from concourse.bass_utils import run_bass_kernel_spmd

N_CORES = 8
B_LOC = 4
CIN = 80
C = 768
D = 64
M = 512
T0 = 1024
T2 = 511
NCH = C // 128
F32 = mybir.dt.float32
FP16 = mybir.dt.float16
U32 = mybir.dt.uint32
RELU = mybir.ActivationFunctionType.Relu
COPY = mybir.ActivationFunctionType.Copy

_CACHED_NC = None


def _build_nc():
    nc = bacc.Bacc("TRN2", target_bir_lowering=False, debug=False,
                   num_devices=N_CORES)

    mels_h = nc.dram_tensor("mels_h", [B_LOC, CIN, T0], FP16, kind="ExternalInput")
    mels_l = nc.dram_tensor("mels_l", [B_LOC, CIN, T0], FP16, kind="ExternalInput")
    wts = {}
    for li, K, cin in ((1, 3, CIN), (3, 4, C)):
        for p in ("h", "l"):
            wts[li, p] = nc.dram_tensor(f"w{li}T{p}", [K, cin, C], FP16,
                                        kind="ExternalInput")
    for li in (2, 4, 5):  # Winograd G-transformed weights
        for p in ("h", "l"):
            wts[li, p] = nc.dram_tensor(f"w{li}G{p}", [4, C, C], FP16,
                                        kind="ExternalInput")
    for p in ("h", "l"):
        wts[6, p] = nc.dram_tensor(f"w6T{p}", [C, D], FP16, kind="ExternalInput")
    bnS = nc.dram_tensor("bnS", [5, NCH, 128, 1], F32, kind="ExternalInput")
    bnB = nc.dram_tensor("bnB", [5, NCH, 128, 1], F32, kind="ExternalInput")
    b6v = nc.dram_tensor("b6v", [D, 1], F32, kind="ExternalInput")
    embT2 = nc.dram_tensor("embT2", [D, M], F32, kind="ExternalInput")
    e2n = nc.dram_tensor("e2n", [128, M], F32, kind="ExternalInput")

    z_out = nc.dram_tensor("z_out", [B_LOC, D, T2], F32, kind="ExternalOutput")
    idx_out = nc.dram_tensor("idx_out", [128, 16], U32, kind="ExternalOutput")

    with tile.TileContext(nc) as tc:
        with (
            tc.tile_pool(name="const", bufs=1) as constp,
            tc.tile_pool(name="dram", bufs=1, space="DRAM") as dramp,
        ):
            bn_s = {}
            bn_b = {}
            for li in range(5):
                for ci in range(NCH):
                    s = constp.tile([128, 1], F32, tag=f"bns_{li}_{ci}")
                    b = constp.tile([128, 1], F32, tag=f"bnb_{li}_{ci}")
                    nc.sync.dma_start(out=s, in_=bnS[li, ci, :, :])
                    nc.sync.dma_start(out=b, in_=bnB[li, ci, :, :])
                    bn_s[li, ci] = s
                    bn_b[li, ci] = b
            b6s = constp.tile([D, 1], F32, tag="b6s")
            nc.sync.dma_start(out=b6s, in_=b6v[:, :])
            embT2s = constp.tile([D, M], F32, tag="embT2s")
            nc.sync.dma_start(out=embT2s, in_=embT2[:, :])
            e2ns = constp.tile([128, M], F32, tag="e2ns")
            nc.sync.dma_start(out=e2ns, in_=e2n[:, :])
            ones64 = constp.tile([D, 1], F32, tag="ones64")
            nc.vector.memset(ones64, 1.0)
            idxacc = constp.tile([128, 16], U32, tag="idxacc")

            # DRAM scratch. fp32 acts feed Winograd layers; fp16 h/l pairs
            # feed the direct layers (L3, L6).
            act = {}
            for li, width, kind in ((1, 1026, "f"), (2, T0, "hl"),
                                    (3, 516, "f"), (4, 516, "f"),
                                    (5, T2, "hl")):
                for b in range(B_LOC):
                    for ci in range(NCH):
                        if kind == "f":
                            act[li, b, ci, "f"] = dramp.tile(
                                [128, width], F32, tag=f"a{li}_{b}_{ci}",
                                name=f"a{li}_{b}_{ci}")
                        else:
                            for p in ("h", "l"):
                                act[li, b, ci, p] = dramp.tile(
                                    [128, width], FP16, tag=f"a{li}_{b}_{ci}{p}",
                                    name=f"a{li}_{b}_{ci}{p}")

            def load_weights(wp, li, K, cin_p, n_cin):
                wt = {}
                for p in ("h", "l"):
                    for k in range(K):
                        for ci in range(n_cin):
                            t = wp.tile([cin_p, C], FP16, tag=f"w{li}{p}_{k}_{ci}",
                                        name=f"w{li}{p}_{k}_{ci}")
                            nc.sync.dma_start(
                                out=t,
                                in_=wts[li, p][k, ci * cin_p:(ci + 1) * cin_p, :])
                            wt[p, k, ci] = t
                return wt

            def emit_out(li, b, co, src_ap_writer, tw, out_lo, pad_lo, pad_hi,
                         out_kind, conv_out):
                """src_ap_writer(dest_ap): emits the ACT op(s) writing the
                activated output rows into dest_ap (width tw)."""
                w_out = pad_lo + tw + pad_hi
                if out_kind == "f":
                    ot = conv_out.tile([128, w_out], F32, tag="of", bufs=3,
                                       name=f"of{li}_{co}")
                    if pad_lo:
                        nc.vector.memset(ot[:, 0:pad_lo], 0.0)
                    if pad_hi:
                        nc.vector.memset(ot[:, pad_lo + tw:w_out], 0.0)
                    src_ap_writer(ot[:, pad_lo:pad_lo + tw])
                    nc.sync.dma_start(
                        out=act[li, b, co, "f"][:, out_lo:out_lo + w_out],
                        in_=ot[:, :])
                else:
                    y32 = conv_out.tile([128, tw], F32, tag="y", bufs=3,
                                        name=f"y{li}_{co}")
                    src_ap_writer(y32[:, :])
                    oh = None
                    for p in ("h", "l"):
                        ot = conv_out.tile([128, w_out], FP16, tag=f"o{p}",
                                           bufs=4, name=f"o{li}_{co}{p}")
                        if pad_lo:
                            nc.vector.memset(ot[:, 0:pad_lo], 0.0)
                        if pad_hi:
                            nc.vector.memset(ot[:, pad_lo + tw:w_out], 0.0)
                        if p == "h":
                            nc.vector.tensor_copy(ot[:, pad_lo:pad_lo + tw],
                                                  y32[:, :])
                            oh = ot
                        else:
                            nc.vector.tensor_sub(ot[:, pad_lo:pad_lo + tw],
                                                 y32[:, :],
                                                 oh[:, pad_lo:pad_lo + tw])
                        nc.sync.dma_start(
                            out=act[li, b, co, p][:, out_lo:out_lo + w_out],
                            in_=ot[:, :])

            def conv_layer(li, K, tiles_spec, get_in, wt, n_cin, psump,
                           conv_in, conv_out, out_kind):
                cin_p = wt["h", 0, 0].shape[0]
                for b in range(B_LOC):
                    for (t0, tw, in_lo, in_w, out_lo, pad_lo, pad_hi,
                         stride) in tiles_spec:
                        ins = {}
                        for ci in range(n_cin):
                            for p in ("h", "l"):
                                it = conv_in.tile([cin_p, in_w], FP16,
                                                  tag=f"in{ci}{p}", bufs=2,
                                                  name=f"cin{li}_{ci}{p}")
                                nc.sync.dma_start(
                                    out=it, in_=get_in(b, ci, p, in_lo, in_w))
                                ins[ci, p] = it
                        for co in range(NCH):
                            ps = psump.tile([128, tw], F32, tag="cps")
                            nmm = n_cin * K * 3
                            i = 0
                            for ci in range(n_cin):
                                for k in range(K):
                                    if stride == 1:
                                        sl = slice(k, k + tw)
                                    else:
                                        sl = slice(k, k + 2 * (tw - 1) + 1, 2)
                                    wh = wt["h", k, ci][:, co * 128:(co + 1) * 128]
                                    wl = wt["l", k, ci][:, co * 128:(co + 1) * 128]
                                    for lhsT, rhs in ((wh, ins[ci, "h"][:, sl]),
                                                      (wh, ins[ci, "l"][:, sl]),
                                                      (wl, ins[ci, "h"][:, sl])):
                                        nc.tensor.matmul(ps[:, :], lhsT, rhs,
                                                         start=(i == 0),
                                                         stop=(i == nmm - 1))
                                        i += 1

                            def write(dest, ps=ps, li=li, co=co):
                                nc.scalar.activation(
                                    out=dest, in_=ps[:, :], func=RELU,
                                    bias=bn_b[li - 1, co][:, :],
                                    scale=bn_s[li - 1, co][:, :])
                            emit_out(li, b, co, write, tw, out_lo, pad_lo,
                                     pad_hi, out_kind, conv_out)

            # D-transform index specs for F(2,3): (in0_off, in1_off, op)
            DSPEC = [(0, 2, "sub"), (1, 2, "add"), (2, 1, "sub"), (1, 3, "sub")]

            def conv_layer_wino(li, tiles_spec, get_in32, wt, psump,
                                conv_in, conv_out, out_kind):
                J = 256
                for b in range(B_LOC):
                    for (t0, tw, in_lo, in_w, out_lo, pad_lo, pad_hi) in tiles_spec:
                        dh = {}
                        dl = {}
                        for ci in range(NCH):
                            x32 = conv_in.tile([128, in_w], F32, tag=f"x{ci}",
                                               bufs=2, name=f"x{li}_{ci}")
                            nc.sync.dma_start(out=x32,
                                              in_=get_in32(b, ci, in_lo, in_w))
                            for i, (a0, a1, op) in enumerate(DSPEC):
                                s0 = x32[:, slice(a0, a0 + 2 * (J - 1) + 1, 2)]
                                s1 = x32[:, slice(a1, a1 + 2 * (J - 1) + 1, 2)]
                                d32 = conv_in.tile([128, J], F32, tag="d32",
                                                   bufs=4, name=f"d32_{li}")
                                if op == "sub":
                                    nc.vector.tensor_sub(d32[:, :], s0, s1)
                                else:
                                    nc.vector.tensor_add(d32[:, :], s0, s1)
                                h = conv_in.tile([128, J], FP16,
                                                 tag=f"dh{ci}_{i}", bufs=2,
                                                 name=f"dh{li}_{ci}_{i}")
                                nc.scalar.activation(out=h[:, :], in_=d32[:, :],
                                                     func=COPY)
                                dsub = conv_in.tile([128, J], F32, tag="dsub",
                                                    bufs=4, name=f"dsub_{li}")
                                nc.vector.tensor_sub(dsub[:, :], d32[:, :],
                                                     h[:, :])
                                l = conv_in.tile([128, J], FP16,
                                                 tag=f"dl{ci}_{i}", bufs=2,
                                                 name=f"dl{li}_{ci}_{i}")
                                nc.vector.tensor_copy(l[:, :], dsub[:, :])
                                dh[ci, i] = h
                                dl[ci, i] = l
                        for co in range(NCH):
                            Mp = psump.tile([128, 4, J], F32, tag="M")
                            for i in range(4):
                                t = 0
                                for ci in range(NCH):
                                    gh = wt["h", i, ci][:, co * 128:(co + 1) * 128]
                                    gl = wt["l", i, ci][:, co * 128:(co + 1) * 128]
                                    for lhsT, rhs in ((gh, dh[ci, i]),
                                                      (gh, dl[ci, i]),
                                                      (gl, dh[ci, i])):
                                        nc.tensor.matmul(
                                            Mp[:, i, :], lhsT, rhs[:, :],
                                            start=(t == 0),
                                            stop=(t == NCH * 3 - 1))
                                        t += 1
                            # assembly: ye = m0+m1+m2 ; yo = m1-m2-m3
                            m1s = conv_out.tile([128, J], F32, tag="m1s",
                                                bufs=2, name=f"m1s{li}")
                            nc.scalar.activation(out=m1s[:, :],
                                                 in_=Mp[:, 1, :], func=COPY)
                            m2s = conv_out.tile([128, J], F32, tag="m2s",
                                                bufs=2, name=f"m2s{li}")
                            nc.scalar.activation(out=m2s[:, :],
                                                 in_=Mp[:, 2, :], func=COPY)
                            ye1 = conv_out.tile([128, J], F32, tag="ye1",
                                                bufs=2, name=f"ye1{li}")
                            nc.vector.tensor_add(ye1[:, :], Mp[:, 0, :],
                                                 m1s[:, :])
                            ye2 = conv_out.tile([128, J], F32, tag="ye2",
                                                bufs=2, name=f"ye2{li}")
                            nc.vector.tensor_add(ye2[:, :], ye1[:, :],
                                                 m2s[:, :])
                            yo1 = conv_out.tile([128, J], F32, tag="yo1",
                                                bufs=2, name=f"yo1{li}")
                            nc.vector.tensor_sub(yo1[:, :], m1s[:, :],
                                                 m2s[:, :])
                            yo2 = conv_out.tile([128, J], F32, tag="yo2",
                                                bufs=2, name=f"yo2{li}")
                            nc.vector.tensor_sub(yo2[:, :], yo1[:, :],
                                                 Mp[:, 3, :])

                            def write(dest, li=li, co=co, ye2=ye2, yo2=yo2,
                                      tw=tw):
                                nc.scalar.activation(
                                    out=dest[:, slice(0, tw, 2)],
                                    in_=ye2[:, 0:(tw + 1) // 2], func=RELU,
                                    bias=bn_b[li - 1, co][:, :],
                                    scale=bn_s[li - 1, co][:, :])
                                nc.scalar.activation(
                                    out=dest[:, slice(1, tw, 2)],
                                    in_=yo2[:, 0:tw // 2], func=RELU,
                                    bias=bn_b[li - 1, co][:, :],
                                    scale=bn_s[li - 1, co][:, :])
                            emit_out(li, b, co, write, tw, out_lo, pad_lo,
                                     pad_hi, out_kind, conv_out)

            # ---- L1: direct conv(80->768, K=3, valid) -> act1 fp32 ---------
            with tc.tile_pool(name="w1p", bufs=1) as wp, \
                 tc.tile_pool(name="c1i", bufs=1) as conv_in, \
                 tc.tile_pool(name="c1o", bufs=1) as conv_out, \
                 tc.tile_pool(name="ps1", bufs=2, space="PSUM") as psump:
                wt = load_weights(wp, 1, 3, CIN, 1)
                spec = [(0, 511, 0, 513, 0, 1, 0, 1),
                        (511, 511, 511, 513, 512, 0, 3, 1)]
                mels_d = {"h": mels_h, "l": mels_l}
                conv_layer(1, 3, spec,
                           lambda b, ci, p, lo, w: mels_d[p][b, :, lo:lo + w],
                           wt, 1, psump, conv_in, conv_out, "f")

            # ---- L2: Winograd conv(768->768) -> act2 h/l -------------------
            with tc.tile_pool(name="w2p", bufs=1) as wp, \
                 tc.tile_pool(name="c2i", bufs=1) as conv_in, \
                 tc.tile_pool(name="c2o", bufs=1) as conv_out, \
                 tc.tile_pool(name="ps2", bufs=2, space="PSUM") as psump:
                wt = load_weights(wp, 2, 4, 128, NCH)
                spec = [(0, 511, 0, 514, 0, 1, 0), (511, 511, 511, 514, 512, 0, 1)]
                conv_layer_wino(2, spec,
                                lambda b, ci, lo, w: act[1, b, ci, "f"][:, lo:lo + w],
                                wt, psump, conv_in, conv_out, "hl")

            # ---- L3: direct strided conv -> act3 fp32 ----------------------
            with tc.tile_pool(name="w3p", bufs=1) as wp, \
                 tc.tile_pool(name="c3i", bufs=1) as conv_in, \
                 tc.tile_pool(name="c3o", bufs=1) as conv_out, \
                 tc.tile_pool(name="ps3", bufs=2, space="PSUM") as psump:
                wt = load_weights(wp, 3, 4, 128, NCH)
                spec = [(0, 511, 0, 1024, 0, 1, 4, 2)]
                conv_layer(3, 4, spec,
                           lambda b, ci, p, lo, w: act[2, b, ci, p][:, lo:lo + w],
                           wt, NCH, psump, conv_in, conv_out, "f")

            # ---- L4: Winograd -> act4 fp32 ---------------------------------
            with tc.tile_pool(name="w4p", bufs=1) as wp, \
                 tc.tile_pool(name="c4i", bufs=1) as conv_in, \
                 tc.tile_pool(name="c4o", bufs=1) as conv_out, \
                 tc.tile_pool(name="ps4", bufs=2, space="PSUM") as psump:
                wt = load_weights(wp, 4, 4, 128, NCH)
                spec = [(0, 511, 0, 514, 0, 1, 4)]
                conv_layer_wino(4, spec,
                                lambda b, ci, lo, w: act[3, b, ci, "f"][:, lo:lo + w],
                                wt, psump, conv_in, conv_out, "f")

            # ---- L5: Winograd -> act5 h/l ----------------------------------
            with tc.tile_pool(name="w5p", bufs=1) as wp, \
                 tc.tile_pool(name="c5i", bufs=1) as conv_in, \
                 tc.tile_pool(name="c5o", bufs=1) as conv_out, \
                 tc.tile_pool(name="ps5", bufs=2, space="PSUM") as psump:
                wt = load_weights(wp, 5, 4, 128, NCH)
                spec = [(0, 511, 0, 514, 0, 0, 0)]
                conv_layer_wino(5, spec,
                                lambda b, ci, lo, w: act[4, b, ci, "f"][:, lo:lo + w],
                                wt, psump, conv_in, conv_out, "hl")

            # ---- L6 (1x1 conv + bias) and VQ ------------------------------
            with tc.tile_pool(name="w6p", bufs=1) as wp, \
                 tc.tile_pool(name="c6i", bufs=1) as conv_in, \
                 tc.tile_pool(name="vq", bufs=2) as vqp, \
                 tc.tile_pool(name="vqsmall", bufs=4) as vqsp, \
                 tc.tile_pool(name="ps6", bufs=2, space="PSUM") as psump:
                wt6 = {}
                for p in ("h", "l"):
                    for ci in range(NCH):
                        t = wp.tile([128, D], FP16, tag=f"w6{p}_{ci}",
                                    name=f"w6{p}_{ci}")
                        nc.sync.dma_start(
                            out=t, in_=wts[6, p][ci * 128:(ci + 1) * 128, :])
                        wt6[p, ci] = t
                for b in range(B_LOC):
                    ins = {}
                    for ci in range(NCH):
                        for p in ("h", "l"):
                            it = conv_in.tile([128, T2], FP16, tag=f"in{ci}{p}",
                                              bufs=2, name=f"cin6_{ci}{p}")
                            nc.sync.dma_start(out=it, in_=act[5, b, ci, p][:, :])
                            ins[ci, p] = it
                    ps = psump.tile([D, T2], F32, tag="zps")
                    nmm = NCH * 3
                    i = 0
                    for ci in range(NCH):
                        for lhsT, rhs in ((wt6["h", ci], ins[ci, "h"]),
                                          (wt6["h", ci], ins[ci, "l"]),
                                          (wt6["l", ci], ins[ci, "h"])):
                            nc.tensor.matmul(ps[:, :], lhsT[:, :], rhs[:, :],
                                             start=(i == 0), stop=(i == nmm - 1))
                            i += 1
                    zb = vqp.tile([D, T2], F32, tag="zb")
                    nc.vector.tensor_scalar_add(zb[:, :], ps[:, :], b6s[:, :])
                    nc.sync.dma_start(out=z_out[b, :, :], in_=zb[:, :])

                    zsq = vqp.tile([D, T2], F32, tag="zsq")
                    nc.vector.tensor_mul(zsq[:, :], zb[:, :], zb[:, :])
                    for c in range(4):
                        c0 = c * 128
                        cs = min(128, T2 - c0)
                        x2p = psump.tile([128, 1], F32, tag="x2p")
                        nc.tensor.matmul(x2p[:cs, :], zsq[:, c0:c0 + cs],
                                         ones64[:, :], start=True, stop=True)
                        x2s = vqsp.tile([128, 1], F32, tag="x2s")
                        nc.vector.tensor_copy(x2s[:cs, :], x2p[:cs, :])
                        scp = psump.tile([128, M], F32, tag="scp")
                        nc.tensor.matmul(scp[:cs, :], zb[:, c0:c0 + cs],
                                         embT2s[:, :], start=True, stop=True)
                        t1 = vqsp.tile([128, M], F32, tag="t1")
                        nc.vector.tensor_scalar_sub(t1[:cs, :], e2ns[:cs, :],
                                                    x2s[:cs, :])
                        nd = vqsp.tile([128, M], F32, tag="nd")
                        nc.vector.tensor_add(nd[:cs, :], t1[:cs, :], scp[:cs, :])
                        mx = vqsp.tile([128, 8], F32, tag="mx")
                        nc.vector.max(mx[:cs, :], nd[:cs, :])
                        mi = vqsp.tile([128, 8], U32, tag="mi")
                        nc.vector.max_index(mi[:cs, :], mx[:cs, :], nd[:cs, :])
                        col = b * 4 + c
                        nc.vector.tensor_copy(idxacc[:cs, col:col + 1],
                                              mi[:cs, 0:1])
            nc.sync.dma_start(out=idx_out[:, :], in_=idxacc[:, :])

    nc.compile()
    return nc


def _get_nc():
    global _CACHED_NC
    if _CACHED_NC is None:
        _CACHED_NC = _build_nc()
    return _CACHED_NC


def _split_hl(x):
    h = x.astype(np.float16)
    l = (x - h.astype(np.float32)).astype(np.float16)
    return np.ascontiguousarray(h), np.ascontiguousarray(l)


def _host_prep(inputs):
    f = np.float32
    out = {}
    for li, key in ((1, "w1"), (3, "w3")):
        wT = np.ascontiguousarray(inputs[key].astype(f).transpose(2, 1, 0))
        out[f"w{li}Th"], out[f"w{li}Tl"] = _split_hl(wT)
    for li, key in ((2, "w2"), (4, "w4"), (5, "w5")):
        wT = inputs[key].astype(f).transpose(2, 1, 0)   # [3, Cin, Cout]
        g = np.empty((4,) + wT.shape[1:], f)
        g[0] = wT[0]
        g[1] = (wT[0] + wT[1] + wT[2]) * f(0.5)
        g[2] = (wT[0] - wT[1] + wT[2]) * f(0.5)
        g[3] = wT[2]
        out[f"w{li}Gh"], out[f"w{li}Gl"] = _split_hl(g)
    w6 = np.ascontiguousarray(inputs["w6"].astype(f)[:, :, 0].T)
    out["w6Th"], out["w6Tl"] = _split_hl(w6)
    gamma = inputs["bn_gamma"].astype(f)
    beta = inputs["bn_beta"].astype(f)
    mean = inputs["bn_mean"].astype(f)
    var = inputs["bn_var"].astype(f)
    inv = gamma / np.sqrt(var + f(1e-5))
    bias = beta - mean * inv
    out["bnS"] = np.ascontiguousarray(inv.reshape(5, NCH, 128, 1))
    out["bnB"] = np.ascontiguousarray(bias.reshape(5, NCH, 128, 1))
    out["b6v"] = np.ascontiguousarray(inputs["b6"].astype(f).reshape(D, 1))
    emb = inputs["embedding"].astype(f)
    out["embT2"] = np.ascontiguousarray(2.0 * emb.T)
    e2 = np.sum(emb.astype(np.float64) ** 2, axis=1).astype(f)
    out["e2n"] = np.ascontiguousarray(np.broadcast_to(-e2[None, :], (128, M)))
    return out, emb


def _make_in_maps(inputs):
    shared, emb = _host_prep(inputs)
    mels = inputs["mels"].astype(np.float32)
    B = mels.shape[0]
    assert B == N_CORES * B_LOC
    in_maps = []
    for c in range(N_CORES):
        m = dict(shared)
        mh, ml = _split_hl(mels[c * B_LOC:(c + 1) * B_LOC])
        m["mels_h"] = mh
        m["mels_l"] = ml
        in_maps.append(m)
    return in_maps, emb


def kernel(**inputs):
    nc = _get_nc()
    in_maps, emb = _make_in_maps(inputs)
    B = N_CORES * B_LOC

    res = run_bass_kernel_spmd(nc, in_maps, core_ids=list(range(N_CORES)))

    z_parts = []
    idx_parts = []
    for c in range(N_CORES):
        r = res.results[c]
        z_parts.append(r["z_out"])
        arr = r["idx_out"]
        loc = np.empty(B_LOC * T2, dtype=np.int64)
        for b in range(B_LOC):
            for ch in range(4):
                c0 = ch * 128
                cs = min(128, T2 - c0)
                loc[b * T2 + c0: b * T2 + c0 + cs] = arr[:cs, b * 4 + ch]
        idx_parts.append(loc)

    z = np.concatenate(z_parts, axis=0).transpose(0, 2, 1)
    z = np.ascontiguousarray(z)
    idx = np.concatenate(idx_parts)

    q = emb[idx].reshape(B, T2, D)
    q_st = z + (q - z)
    diff = z.astype(np.float64) - q.astype(np.float64)
    loss = np.float32(0.25 * np.mean(diff * diff))
    counts = np.bincount(idx, minlength=M).astype(np.float64)
    avg = counts / idx.shape[0]
    perplexity = np.float32(np.exp(-np.sum(avg * np.log(avg + 1e-10))))
    return q_st, loss, perplexity


# revision 14
# speedup vs baseline: 1.0012x; 1.0012x over previous
"""Trainium2 Bass kernel for nn_Encoder (conv stack + VQ codebook).

Reference computation (fp32):
  x = mels [32, 80, 1024]
  5x (conv1d + batchnorm-affine + relu), 1x 1x1-conv + bias  -> z [32, 64, 511]
  VQ: nearest codebook row (squared L2, 512 codes, D=64) -> q_st, loss, perplexity

Sharding: data-parallel over batch across 8 NeuronCores (4 batches/core);
conv weights + codebook replicated.

Speed tricks (all verified to keep VQ argmins identical to the fp32 ref):
- fp16 hi/lo 3-term matmuls (W=Wh+Wl, X=Xh+Xl; Y ~= Wh.Xh + Wh.Xl + Wl.Xh):
  1 PE cycle/row instead of fp32's 4, output rel err ~3e-6 (the PE supports
  fp16 subnormals, so the lo parts need no scaling).
- Winograd F(2,3) on the stride-1 K=3 768x768 layers (L2/L4/L5): 4 transformed
  256-col matmuls per output pair instead of 3x511 direct columns (1.5x fewer
  PE cycles), transforms on DVE/ACT in fp32.
- The VQ distance computation stays fully fp32 and mirrors the reference's
  operation order; host finishes gather/loss/perplexity exactly as the
  reference does.
"""

import numpy as np

import concourse.bass as bass
import concourse.tile as tile
from concourse import bacc, mybir
from concourse.bass_utils import run_bass_kernel_spmd

N_CORES = 8
B_LOC = 4
CIN = 80
C = 768
D = 64
M = 512
T0 = 1024
T2 = 511
NCH = C // 128
F32 = mybir.dt.float32
FP16 = mybir.dt.float16
U32 = mybir.dt.uint32
RELU = mybir.ActivationFunctionType.Relu
COPY = mybir.ActivationFunctionType.Copy

_CACHED_NC = None


def _build_nc():
    nc = bacc.Bacc("TRN2", target_bir_lowering=False, debug=False,
                   num_devices=N_CORES)

    mels_h = nc.dram_tensor("mels_h", [B_LOC, CIN, T0], FP16, kind="ExternalInput")
    mels_l = nc.dram_tensor("mels_l", [B_LOC, CIN, T0], FP16, kind="ExternalInput")
    wts = {}
    for li, K, cin in ((1, 3, CIN), (3, 4, C)):
        for p in ("h", "l"):
            wts[li, p] = nc.dram_tensor(f"w{li}T{p}", [K, cin, C], FP16,
                                        kind="ExternalInput")
    for li in (2, 4, 5):  # Winograd G-transformed weights
        for p in ("h", "l"):
            wts[li, p] = nc.dram_tensor(f"w{li}G{p}", [4, C, C], FP16,
                                        kind="ExternalInput")
    for p in ("h", "l"):
        wts[6, p] = nc.dram_tensor(f"w6T{p}", [C, D], FP16, kind="ExternalInput")
    bnS = nc.dram_tensor("bnS", [5, NCH, 128, 1], F32, kind="ExternalInput")
    bnB = nc.dram_tensor("bnB", [5, NCH, 128, 1], F32, kind="ExternalInput")
    b6v = nc.dram_tensor("b6v", [D, 1], F32, kind="ExternalInput")
    embT2 = nc.dram_tensor("embT2", [D, M], F32, kind="ExternalInput")
    e2n = nc.dram_tensor("e2n", [128, M], F32, kind="ExternalInput")

    z_out = nc.dram_tensor("z_out", [B_LOC, D, T2], F32, kind="ExternalOutput")
    idx_out = nc.dram_tensor("idx_out", [128, 16], U32, kind="ExternalOutput")

    with tile.TileContext(nc) as tc:
        with (
            tc.tile_pool(name="const", bufs=1) as constp,
            tc.tile_pool(name="dram", bufs=1, space="DRAM") as dramp,
        ):
            bn_s = {}
            bn_b = {}
            for li in range(5):
                for ci in range(NCH):
                    s = constp.tile([128, 1], F32, tag=f"bns_{li}_{ci}")
                    b = constp.tile([128, 1], F32, tag=f"bnb_{li}_{ci}")
                    nc.sync.dma_start(out=s, in_=bnS[li, ci, :, :])
                    nc.sync.dma_start(out=b, in_=bnB[li, ci, :, :])
                    bn_s[li, ci] = s
                    bn_b[li, ci] = b
            b6s = constp.tile([D, 1], F32, tag="b6s")
            nc.sync.dma_start(out=b6s, in_=b6v[:, :])
            embT2s = constp.tile([D, M], F32, tag="embT2s")
            nc.sync.dma_start(out=embT2s, in_=embT2[:, :])
            e2ns = constp.tile([128, M], F32, tag="e2ns")
            nc.sync.dma_start(out=e2ns, in_=e2n[:, :])
            ones64 = constp.tile([D, 1], F32, tag="ones64")
            nc.vector.memset(ones64, 1.0)
            idxacc = constp.tile([128, 16], U32, tag="idxacc")

            # DRAM scratch. fp32 acts feed Winograd layers; fp16 h/l pairs
            # feed the direct layers (L3, L6).
            act = {}
            for li, width, kind in ((1, 1026, "f"), (2, T0, "hl"),
                                    (3, 516, "f"), (4, 516, "f"),
                                    (5, T2, "hl")):
                for b in range(B_LOC):
                    for ci in range(NCH):
                        if kind == "f":
                            act[li, b, ci, "f"] = dramp.tile(
                                [128, width], F32, tag=f"a{li}_{b}_{ci}",
                                name=f"a{li}_{b}_{ci}")
                        else:
                            for p in ("h", "l"):
                                act[li, b, ci, p] = dramp.tile(
                                    [128, width], FP16, tag=f"a{li}_{b}_{ci}{p}",
                                    name=f"a{li}_{b}_{ci}{p}")

            def load_weights(wp, li, K, cin_p, n_cin):
                wt = {}
                for p in ("h", "l"):
                    for k in range(K):
                        for ci in range(n_cin):
                            t = wp.tile([cin_p, C], FP16, tag=f"w{li}{p}_{k}_{ci}",
                                        name=f"w{li}{p}_{k}_{ci}")
                            nc.sync.dma_start(
                                out=t,
                                in_=wts[li, p][k, ci * cin_p:(ci + 1) * cin_p, :])
                            wt[p, k, ci] = t
                return wt

            def emit_out(li, b, co, src_ap_writer, tw, out_lo, pad_lo, pad_hi,
                         out_kind, conv_out):
                """src_ap_writer(dest_ap): emits the ACT op(s) writing the
                activated output rows into dest_ap (width tw)."""
                w_out = pad_lo + tw + pad_hi
                if out_kind == "f":
                    ot = conv_out.tile([128, w_out], F32, tag="of", bufs=3,
                                       name=f"of{li}_{co}")
                    if pad_lo:
                        nc.vector.memset(ot[:, 0:pad_lo], 0.0)
                    if pad_hi:
                        nc.vector.memset(ot[:, pad_lo + tw:w_out], 0.0)
                    src_ap_writer(ot[:, pad_lo:pad_lo + tw])
                    nc.sync.dma_start(
                        out=act[li, b, co, "f"][:, out_lo:out_lo + w_out],
                        in_=ot[:, :])
                else:
                    y32 = conv_out.tile([128, tw], F32, tag="y", bufs=3,
                                        name=f"y{li}_{co}")
                    src_ap_writer(y32[:, :])
                    oh = None
                    for p in ("h", "l"):
                        ot = conv_out.tile([128, w_out], FP16, tag=f"o{p}",
                                           bufs=4, name=f"o{li}_{co}{p}")
                        if pad_lo:
                            nc.vector.memset(ot[:, 0:pad_lo], 0.0)
                        if pad_hi:
                            nc.vector.memset(ot[:, pad_lo + tw:w_out], 0.0)
                        if p == "h":
                            nc.vector.tensor_copy(ot[:, pad_lo:pad_lo + tw],
                                                  y32[:, :])
                            oh = ot
                        else:
                            nc.vector.tensor_sub(ot[:, pad_lo:pad_lo + tw],
                                                 y32[:, :],
                                                 oh[:, pad_lo:pad_lo + tw])
                        nc.sync.dma_start(
                            out=act[li, b, co, p][:, out_lo:out_lo + w_out],
                            in_=ot[:, :])

            def conv_layer(li, K, tiles_spec, get_in, wt, n_cin, psump,
                           conv_in, conv_out, out_kind):
                cin_p = wt["h", 0, 0].shape[0]
                for b in range(B_LOC):
                    for (t0, tw, in_lo, in_w, out_lo, pad_lo, pad_hi,
                         stride) in tiles_spec:
                        ins = {}
                        for ci in range(n_cin):
                            for p in ("h", "l"):
                                it = conv_in.tile([cin_p, in_w], FP16,
                                                  tag=f"in{ci}{p}", bufs=2,
                                                  name=f"cin{li}_{ci}{p}")
                                nc.sync.dma_start(
                                    out=it, in_=get_in(b, ci, p, in_lo, in_w))
                                ins[ci, p] = it
                        for co in range(NCH):
                            ps = psump.tile([128, tw], F32, tag="cps")
                            nmm = n_cin * K * 3
                            i = 0
                            for ci in range(n_cin):
                                for k in range(K):
                                    if stride == 1:
                                        sl = slice(k, k + tw)
                                    else:
                                        sl = slice(k, k + 2 * (tw - 1) + 1, 2)
                                    wh = wt["h", k, ci][:, co * 128:(co + 1) * 128]
                                    wl = wt["l", k, ci][:, co * 128:(co + 1) * 128]
                                    for lhsT, rhs in ((wh, ins[ci, "h"][:, sl]),
                                                      (wh, ins[ci, "l"][:, sl]),
                                                      (wl, ins[ci, "h"][:, sl])):
                                        nc.tensor.matmul(ps[:, :], lhsT, rhs,
                                                         start=(i == 0),
                                                         stop=(i == nmm - 1))
                                        i += 1

                            def write(dest, ps=ps, li=li, co=co):
                                nc.scalar.activation(
                                    out=dest, in_=ps[:, :], func=RELU,
                                    bias=bn_b[li - 1, co][:, :],
                                    scale=bn_s[li - 1, co][:, :])
                            emit_out(li, b, co, write, tw, out_lo, pad_lo,
                                     pad_hi, out_kind, conv_out)

            # D-transform index specs for F(2,3): (in0_off, in1_off, op)
            DSPEC = [(0, 2, "sub"), (1, 2, "add"), (2, 1, "sub"), (1, 3, "sub")]

            def conv_layer_wino(li, tiles_spec, get_in32, wt, psump,
                                conv_in, conv_out, out_kind):
                J = 256
                for b in range(B_LOC):
                    for (t0, tw, in_lo, in_w, out_lo, pad_lo, pad_hi) in tiles_spec:
                        dh = {}
                        dl = {}
                        for ci in range(NCH):
                            x32 = conv_in.tile([128, in_w], F32, tag=f"x{ci}",
                                               bufs=2, name=f"x{li}_{ci}")
                            nc.sync.dma_start(out=x32,
                                              in_=get_in32(b, ci, in_lo, in_w))
                            for i, (a0, a1, op) in enumerate(DSPEC):
                                s0 = x32[:, slice(a0, a0 + 2 * (J - 1) + 1, 2)]
                                s1 = x32[:, slice(a1, a1 + 2 * (J - 1) + 1, 2)]
                                d32 = conv_in.tile([128, J], F32, tag="d32",
                                                   bufs=4, name=f"d32_{li}")
                                if op == "sub":
                                    nc.vector.tensor_sub(d32[:, :], s0, s1)
                                else:
                                    nc.vector.tensor_add(d32[:, :], s0, s1)
                                h = conv_in.tile([128, J], FP16,
                                                 tag=f"dh{ci}_{i}", bufs=2,
                                                 name=f"dh{li}_{ci}_{i}")
                                nc.scalar.activation(out=h[:, :], in_=d32[:, :],
                                                     func=COPY)
                                dsub = conv_in.tile([128, J], F32, tag="dsub",
                                                    bufs=4, name=f"dsub_{li}")
                                nc.vector.tensor_sub(dsub[:, :], d32[:, :],
                                                     h[:, :])
                                l = conv_in.tile([128, J], FP16,
                                                 tag=f"dl{ci}_{i}", bufs=2,
                                                 name=f"dl{li}_{ci}_{i}")
                                nc.vector.tensor_copy(l[:, :], dsub[:, :])
                                dh[ci, i] = h
                                dl[ci, i] = l
                        for co in range(NCH):
                            Mp = psump.tile([128, 4, J], F32, tag="M")
                            for i in range(4):
                                t = 0
                                for ci in range(NCH):
                                    gh = wt["h", i, ci][:, co * 128:(co + 1) * 128]
                                    gl = wt["l", i, ci][:, co * 128:(co + 1) * 128]
                                    for lhsT, rhs in ((gh, dh[ci, i]),
                                                      (gh, dl[ci, i]),
                                                      (gl, dh[ci, i])):
                                        nc.tensor.matmul(
                                            Mp[:, i, :], lhsT, rhs[:, :],
                                            start=(t == 0),
                                            stop=(t == NCH * 3 - 1))
                                        t += 1
                            # assembly: ye = m0+m1+m2 ; yo = m1-m2-m3
                            m1s = conv_out.tile([128, J], F32, tag="m1s",
                                                bufs=2, name=f"m1s{li}")
                            nc.scalar.activation(out=m1s[:, :],
                                                 in_=Mp[:, 1, :], func=COPY)
                            m2s = conv_out.tile([128, J], F32, tag="m2s",
                                                bufs=2, name=f"m2s{li}")
                            nc.scalar.activation(out=m2s[:, :],
                                                 in_=Mp[:, 2, :], func=COPY)
                            ye1 = conv_out.tile([128, J], F32, tag="ye1",
                                                bufs=2, name=f"ye1{li}")
                            nc.vector.tensor_add(ye1[:, :], Mp[:, 0, :],
                                                 m1s[:, :])
                            ye2 = conv_out.tile([128, J], F32, tag="ye2",
                                                bufs=2, name=f"ye2{li}")
                            nc.vector.tensor_add(ye2[:, :], ye1[:, :],
                                                 m2s[:, :])
                            yo1 = conv_out.tile([128, J], F32, tag="yo1",
                                                bufs=2, name=f"yo1{li}")
                            nc.vector.tensor_sub(yo1[:, :], m1s[:, :],
                                                 m2s[:, :])
                            yo2 = conv_out.tile([128, J], F32, tag="yo2",
                                                bufs=2, name=f"yo2{li}")
                            nc.vector.tensor_sub(yo2[:, :], yo1[:, :],
                                                 Mp[:, 3, :])

                            def write(dest, li=li, co=co, ye2=ye2, yo2=yo2,
                                      tw=tw):
                                nc.scalar.activation(
                                    out=dest[:, slice(0, tw, 2)],
                                    in_=ye2[:, 0:(tw + 1) // 2], func=RELU,
                                    bias=bn_b[li - 1, co][:, :],
                                    scale=bn_s[li - 1, co][:, :])
                                nc.scalar.activation(
                                    out=dest[:, slice(1, tw, 2)],
                                    in_=yo2[:, 0:tw // 2], func=RELU,
                                    bias=bn_b[li - 1, co][:, :],
                                    scale=bn_s[li - 1, co][:, :])
                            emit_out(li, b, co, write, tw, out_lo, pad_lo,
                                     pad_hi, out_kind, conv_out)

            # ---- L1: direct conv(80->768, K=3, valid) -> act1 fp32 ---------
            with tc.tile_pool(name="w1p", bufs=1) as wp, \
                 tc.tile_pool(name="c1i", bufs=1) as conv_in, \
                 tc.tile_pool(name="c1o", bufs=1) as conv_out, \
                 tc.tile_pool(name="ps1", bufs=2, space="PSUM") as psump:
                wt = load_weights(wp, 1, 3, CIN, 1)
                spec = [(0, 511, 0, 513, 0, 1, 0, 1),
                        (511, 511, 511, 513, 512, 0, 3, 1)]
                mels_d = {"h": mels_h, "l": mels_l}
                conv_layer(1, 3, spec,
                           lambda b, ci, p, lo, w: mels_d[p][b, :, lo:lo + w],
                           wt, 1, psump, conv_in, conv_out, "f")

            # ---- L2: Winograd conv(768->768) -> act2 h/l -------------------
            with tc.tile_pool(name="w2p", bufs=1) as wp, \
                 tc.tile_pool(name="c2i", bufs=1) as conv_in, \
                 tc.tile_pool(name="c2o", bufs=1) as conv_out, \
                 tc.tile_pool(name="ps2", bufs=2, space="PSUM") as psump:
                wt = load_weights(wp, 2, 4, 128, NCH)
                spec = [(0, 511, 0, 514, 0, 1, 0), (511, 511, 511, 514, 512, 0, 1)]
                conv_layer_wino(2, spec,
                                lambda b, ci, lo, w: act[1, b, ci, "f"][:, lo:lo + w],
                                wt, psump, conv_in, conv_out, "hl")

            # ---- L3: direct strided conv -> act3 fp32 ----------------------
            with tc.tile_pool(name="w3p", bufs=1) as wp, \
                 tc.tile_pool(name="c3i", bufs=1) as conv_in, \
                 tc.tile_pool(name="c3o", bufs=1) as conv_out, \
                 tc.tile_pool(name="ps3", bufs=2, space="PSUM") as psump:
                wt = load_weights(wp, 3, 4, 128, NCH)
                spec = [(0, 511, 0, 1024, 0, 1, 4, 2)]
                conv_layer(3, 4, spec,
                           lambda b, ci, p, lo, w: act[2, b, ci, p][:, lo:lo + w],
                           wt, NCH, psump, conv_in, conv_out, "f")

            # ---- L4: Winograd -> act4 fp32 ---------------------------------
            with tc.tile_pool(name="w4p", bufs=1) as wp, \
                 tc.tile_pool(name="c4i", bufs=1) as conv_in, \
                 tc.tile_pool(name="c4o", bufs=1) as conv_out, \
                 tc.tile_pool(name="ps4", bufs=2, space="PSUM") as psump:
                wt = load_weights(wp, 4, 4, 128, NCH)
                spec = [(0, 511, 0, 514, 0, 1, 4)]
                conv_layer_wino(4, spec,
                                lambda b, ci, lo, w: act[3, b, ci, "f"][:, lo:lo + w],
                                wt, psump, conv_in, conv_out, "f")

            # ---- L5: Winograd -> act5 h/l ----------------------------------
            with tc.tile_pool(name="w5p", bufs=1) as wp, \
                 tc.tile_pool(name="c5i", bufs=1) as conv_in, \
                 tc.tile_pool(name="c5o", bufs=1) as conv_out, \
                 tc.tile_pool(name="ps5", bufs=2, space="PSUM") as psump:
                wt = load_weights(wp, 5, 4, 128, NCH)
                spec = [(0, 511, 0, 514, 0, 0, 0)]
                conv_layer_wino(5, spec,
                                lambda b, ci, lo, w: act[4, b, ci, "f"][:, lo:lo + w],
                                wt, psump, conv_in, conv_out, "hl")

            # ---- L6 (1x1 conv + bias) and VQ ------------------------------
            with tc.tile_pool(name="w6p", bufs=1) as wp, \
                 tc.tile_pool(name="c6i", bufs=1) as conv_in, \
                 tc.tile_pool(name="vq", bufs=2) as vqp, \
                 tc.tile_pool(name="vqsmall", bufs=4) as vqsp, \
                 tc.tile_pool(name="ps6", bufs=2, space="PSUM") as psump:
                wt6 = {}
                for p in ("h", "l"):
                    for ci in range(NCH):
                        t = wp.tile([128, D], FP16, tag=f"w6{p}_{ci}",
                                    name=f"w6{p}_{ci}")
                        nc.sync.dma_start(
                            out=t, in_=wts[6, p][ci * 128:(ci + 1) * 128, :])
                        wt6[p, ci] = t
                for b in range(B_LOC):
                    ins = {}
                    for ci in range(NCH):
                        for p in ("h", "l"):
                            it = conv_in.tile([128, T2], FP16, tag=f"in{ci}{p}",
                                              bufs=2, name=f"cin6_{ci}{p}")
                            nc.sync.dma_start(out=it, in_=act[5, b, ci, p][:, :])
                            ins[ci, p] = it
                    ps = psump.tile([D, T2], F32, tag="zps")
                    nmm = NCH * 3
                    i = 0
                    for ci in range(NCH):
                        for lhsT, rhs in ((wt6["h", ci], ins[ci, "h"]),
                                          (wt6["h", ci], ins[ci, "l"]),
                                          (wt6["l", ci], ins[ci, "h"])):
                            nc.tensor.matmul(ps[:, :], lhsT[:, :], rhs[:, :],
                                             start=(i == 0), stop=(i == nmm - 1))
                            i += 1
                    zb = vqp.tile([D, T2], F32, tag="zb")
                    nc.vector.tensor_scalar_add(zb[:, :], ps[:, :], b6s[:, :])
                    nc.sync.dma_start(out=z_out[b, :, :], in_=zb[:, :])

                    zsq = vqp.tile([D, T2], F32, tag="zsq")
                    nc.vector.tensor_mul(zsq[:, :], zb[:, :], zb[:, :])
                    for c in range(4):
                        c0 = c * 128
                        cs = min(128, T2 - c0)
                        x2p = psump.tile([128, 1], F32, tag="x2p")
                        nc.tensor.matmul(x2p[:cs, :], zsq[:, c0:c0 + cs],
                                         ones64[:, :], start=True, stop=True)
                        x2s = vqsp.tile([128, 1], F32, tag="x2s")
                        nc.vector.tensor_copy(x2s[:cs, :], x2p[:cs, :])
                        scp = psump.tile([128, M], F32, tag="scp")
                        nc.tensor.matmul(scp[:cs, :], zb[:, c0:c0 + cs],
                                         embT2s[:, :], start=True, stop=True)
                        t1 = vqsp.tile([128, M], F32, tag="t1")
                        nc.vector.tensor_scalar_sub(t1[:cs, :], e2ns[:cs, :],
                                                    x2s[:cs, :])
                        nd = vqsp.tile([128, M], F32, tag="nd")
                        nc.vector.tensor_add(nd[:cs, :], t1[:cs, :], scp[:cs, :])
                        mx = vqsp.tile([128, 8], F32, tag="mx")
                        nc.vector.max(mx[:cs, :], nd[:cs, :])
                        mi = vqsp.tile([128, 8], U32, tag="mi")
                        nc.vector.max_index(mi[:cs, :], mx[:cs, :], nd[:cs, :])
                        col = b * 4 + c
                        nc.vector.tensor_copy(idxacc[:cs, col:col + 1],
                                              mi[:cs, 0:1])
            nc.sync.dma_start(out=idx_out[:, :], in_=idxacc[:, :])

    nc.compile()
    return nc


def _get_nc():
    global _CACHED_NC
    if _CACHED_NC is None:
        _CACHED_NC = _build_nc()
    return _CACHED_NC


def _split_hl(x):
    h = x.astype(np.float16)
    l = (x - h.astype(np.float32)).astype(np.float16)
    return np.ascontiguousarray(h), np.ascontiguousarray(l)


def _host_prep(inputs):
    f = np.float32
    out = {}
    for li, key in ((1, "w1"), (3, "w3")):
        wT = np.ascontiguousarray(inputs[key].astype(f).transpose(2, 1, 0))
        out[f"w{li}Th"], out[f"w{li}Tl"] = _split_hl(wT)
    for li, key in ((2, "w2"), (4, "w4"), (5, "w5")):
        wT = inputs[key].astype(f).transpose(2, 1, 0)   # [3, Cin, Cout]
        g = np.empty((4,) + wT.shape[1:], f)
        g[0] = wT[0]
        g[1] = (wT[0] + wT[1] + wT[2]) * f(0.5)
        g[2] = (wT[0] - wT[1] + wT[2]) * f(0.5)
        g[3] = wT[2]
        out[f"w{li}Gh"], out[f"w{li}Gl"] = _split_hl(g)
    w6 = np.ascontiguousarray(inputs["w6"].astype(f)[:, :, 0].T)
    out["w6Th"], out["w6Tl"] = _split_hl(w6)
    gamma = inputs["bn_gamma"].astype(f)
    beta = inputs["bn_beta"].astype(f)
    mean = inputs["bn_mean"].astype(f)
    var = inputs["bn_var"].astype(f)
    inv = gamma / np.sqrt(var + f(1e-5))
    bias = beta - mean * inv
    out["bnS"] = np.ascontiguousarray(inv.reshape(5, NCH, 128, 1))
    out["bnB"] = np.ascontiguousarray(bias.reshape(5, NCH, 128, 1))
    out["b6v"] = np.ascontiguousarray(inputs["b6"].astype(f).reshape(D, 1))
    emb = inputs["embedding"].astype(f)
    out["embT2"] = np.ascontiguousarray(2.0 * emb.T)
    e2 = np.sum(emb.astype(np.float64) ** 2, axis=1).astype(f)
    out["e2n"] = np.ascontiguousarray(np.broadcast_to(-e2[None, :], (128, M)))
    return out, emb


def _make_in_maps(inputs):
    shared, emb = _host_prep(inputs)
    mels = inputs["mels"].astype(np.float32)
    B = mels.shape[0]
    assert B == N_CORES * B_LOC
    in_maps = []
    for c in range(N_CORES):
        m = dict(shared)
        mh, ml = _split_hl(mels[c * B_LOC:(c + 1) * B_LOC])
        m["mels_h"] = mh
        m["mels_l"] = ml
        in_maps.append(m)
    return in_maps, emb


def kernel(**inputs):
    nc = _get_nc()
    in_maps, emb = _make_in_maps(inputs)
    B = N_CORES * B_LOC

    res = run_bass_kernel_spmd(nc, in_maps, core_ids=list(range(N_CORES)))

    z_parts = []
    idx_parts = []
    for c in range(N_CORES):
        r = res.results[c]
        z_parts.append(r["z_out"])
        arr = r["idx_out"]
        loc = np.empty(B_LOC * T2, dtype=np.int64)
        for b in range(B_LOC):
            for ch in range(4):
                c0 = ch * 128
                cs = min(128, T2 - c0)
                loc[b * T2 + c0: b * T2 + c0 + cs] = arr[:cs, b * 4 + ch]
        idx_parts.append(loc)

    z = np.concatenate(z_parts, axis=0).transpose(0, 2, 1)
    z = np.ascontiguousarray(z)
    idx = np.concatenate(idx_parts)

    q = emb[idx].reshape(B, T2, D)
    q_st = z + (q - z)
    diff = z.astype(np.float64) - q.astype(np.float64)
    loss = np.float32(0.25 * np.mean(diff * diff))
    counts = np.bincount(idx, minlength=M).astype(np.float64)
    avg = counts / idx.shape[0]
    perplexity = np.float32(np.exp(-np.sum(avg * np.log(avg + 1e-10))))
    return q_st, loss, perplexity


# revision 15
# speedup vs baseline: 1.0148x; 1.0135x over previous
"""Trainium2 Bass kernel for nn_Encoder (conv stack + VQ codebook).

Reference computation (fp32):
  x = mels [32, 80, 1024]
  5x (conv1d + batchnorm-affine + relu), 1x 1x1-conv + bias  -> z [32, 64, 511]
  VQ: nearest codebook row (squared L2, 512 codes, D=64) -> q_st, loss, perplexity

Sharding: data-parallel over batch across 8 NeuronCores (4 batches/core);
conv weights + codebook replicated.

Speed tricks (all verified to keep VQ argmins identical to the fp32 ref):
- fp16 hi/lo 3-term matmuls (W=Wh+Wl, X=Xh+Xl; Y ~= Wh.Xh + Wh.Xl + Wl.Xh):
  1 PE cycle/row instead of fp32's 4, output rel err ~3e-6 (the PE supports
  fp16 subnormals, so the lo parts need no scaling).
- Winograd F(2,3) on the stride-1 K=3 768x768 layers (L2/L4/L5): 4 transformed
  256-col matmuls per output pair instead of 3x511 direct columns (1.5x fewer
  PE cycles), transforms on DVE/ACT in fp32.
- The VQ distance computation stays fully fp32 and mirrors the reference's
  operation order; host finishes gather/loss/perplexity exactly as the
  reference does.
"""

import numpy as np

import concourse.bass as bass
import concourse.tile as tile
from concourse import bacc, mybir
from concourse.bass_utils import run_bass_kernel_spmd

N_CORES = 8
B_LOC = 4
CIN = 80
C = 768
D = 64
M = 512
T0 = 1024
T2 = 511
NCH = C // 128
F32 = mybir.dt.float32
FP16 = mybir.dt.float16
U32 = mybir.dt.uint32
RELU = mybir.ActivationFunctionType.Relu
COPY = mybir.ActivationFunctionType.Copy

_CACHED_NC = None


def _build_nc():
    nc = bacc.Bacc("TRN2", target_bir_lowering=False, debug=False,
                   num_devices=N_CORES)

    mels_h = nc.dram_tensor("mels_h", [B_LOC, CIN, T0], FP16, kind="ExternalInput")
    mels_l = nc.dram_tensor("mels_l", [B_LOC, CIN, T0], FP16, kind="ExternalInput")
    wts = {}
    for li, K, cin in ((1, 3, CIN), (3, 4, C)):
        for p in ("h", "l"):
            wts[li, p] = nc.dram_tensor(f"w{li}T{p}", [K, cin, C], FP16,
                                        kind="ExternalInput")
    for li in (2, 4, 5):  # Winograd G-transformed weights
        for p in ("h", "l"):
            wts[li, p] = nc.dram_tensor(f"w{li}G{p}", [4, C, C], FP16,
                                        kind="ExternalInput")
    for p in ("h", "l"):
        wts[6, p] = nc.dram_tensor(f"w6T{p}", [C, D], FP16, kind="ExternalInput")
    bnS = nc.dram_tensor("bnS", [5, NCH, 128, 1], F32, kind="ExternalInput")
    bnB = nc.dram_tensor("bnB", [5, NCH, 128, 1], F32, kind="ExternalInput")
    b6v = nc.dram_tensor("b6v", [D, 1], F32, kind="ExternalInput")
    embT2 = nc.dram_tensor("embT2", [D, M], F32, kind="ExternalInput")
    e2n = nc.dram_tensor("e2n", [128, M], F32, kind="ExternalInput")

    z_out = nc.dram_tensor("z_out", [B_LOC, D, T2], F32, kind="ExternalOutput")
    idx_out = nc.dram_tensor("idx_out", [128, 16], U32, kind="ExternalOutput")

    with tile.TileContext(nc) as tc:
        with (
            tc.tile_pool(name="const", bufs=1) as constp,
            tc.tile_pool(name="dram", bufs=1, space="DRAM") as dramp,
        ):
            # Const tiles are allocated here, but their (slow, strided) DMAs
            # are emitted after L1's weight loads, on the gpsimd SWDGE queues,
            # so they don't delay the first conv matmuls.
            bn_s = {}
            bn_b = {}
            for li in range(5):
                for ci in range(NCH):
                    s = constp.tile([128, 1], F32, tag=f"bns_{li}_{ci}")
                    b = constp.tile([128, 1], F32, tag=f"bnb_{li}_{ci}")
                    bn_s[li, ci] = s
                    bn_b[li, ci] = b
            b6s = constp.tile([D, 1], F32, tag="b6s")
            embT2s = constp.tile([D, M], F32, tag="embT2s")
            e2ns = constp.tile([128, M], F32, tag="e2ns")
            ones64 = constp.tile([D, 1], F32, tag="ones64")
            nc.vector.memset(ones64, 1.0)
            idxacc = constp.tile([128, 16], U32, tag="idxacc")

            def emit_const_dmas():
                for li in range(5):
                    for ci in range(NCH):
                        nc.gpsimd.dma_start(out=bn_s[li, ci],
                                            in_=bnS[li, ci, :, :])
                        nc.gpsimd.dma_start(out=bn_b[li, ci],
                                            in_=bnB[li, ci, :, :])
                nc.gpsimd.dma_start(out=b6s, in_=b6v[:, :])
                nc.gpsimd.dma_start(out=embT2s, in_=embT2[:, :])
                nc.gpsimd.dma_start(out=e2ns, in_=e2n[:, :])

            # DRAM scratch. fp32 acts feed Winograd layers; fp16 h/l pairs
            # feed the direct layers (L3, L6).
            act = {}
            for li, width, kind in ((1, 1026, "f"), (2, T0, "hl"),
                                    (3, 516, "f"), (4, 516, "f"),
                                    (5, T2, "hl")):
                for b in range(B_LOC):
                    for ci in range(NCH):
                        if kind == "f":
                            act[li, b, ci, "f"] = dramp.tile(
                                [128, width], F32, tag=f"a{li}_{b}_{ci}",
                                name=f"a{li}_{b}_{ci}")
                        else:
                            for p in ("h", "l"):
                                act[li, b, ci, p] = dramp.tile(
                                    [128, width], FP16, tag=f"a{li}_{b}_{ci}{p}",
                                    name=f"a{li}_{b}_{ci}{p}")

            def load_weights(wp, li, K, cin_p, n_cin):
                wt = {}
                for p in ("h", "l"):
                    for k in range(K):
                        for ci in range(n_cin):
                            t = wp.tile([cin_p, C], FP16, tag=f"w{li}{p}_{k}_{ci}",
                                        name=f"w{li}{p}_{k}_{ci}")
                            nc.sync.dma_start(
                                out=t,
                                in_=wts[li, p][k, ci * cin_p:(ci + 1) * cin_p, :])
                            wt[p, k, ci] = t
                return wt

            def emit_out(li, b, co, src_ap_writer, tw, out_lo, pad_lo, pad_hi,
                         out_kind, conv_out):
                """src_ap_writer(dest_ap): emits the ACT op(s) writing the
                activated output rows into dest_ap (width tw)."""
                w_out = pad_lo + tw + pad_hi
                if out_kind == "f":
                    ot = conv_out.tile([128, w_out], F32, tag="of", bufs=3,
                                       name=f"of{li}_{co}")
                    if pad_lo:
                        nc.vector.memset(ot[:, 0:pad_lo], 0.0)
                    if pad_hi:
                        nc.vector.memset(ot[:, pad_lo + tw:w_out], 0.0)
                    src_ap_writer(ot[:, pad_lo:pad_lo + tw])
                    nc.sync.dma_start(
                        out=act[li, b, co, "f"][:, out_lo:out_lo + w_out],
                        in_=ot[:, :])
                else:
                    y32 = conv_out.tile([128, tw], F32, tag="y", bufs=3,
                                        name=f"y{li}_{co}")
                    src_ap_writer(y32[:, :])
                    oh = None
                    for p in ("h", "l"):
                        ot = conv_out.tile([128, w_out], FP16, tag=f"o{p}",
                                           bufs=4, name=f"o{li}_{co}{p}")
                        if pad_lo:
                            nc.vector.memset(ot[:, 0:pad_lo], 0.0)
                        if pad_hi:
                            nc.vector.memset(ot[:, pad_lo + tw:w_out], 0.0)
                        if p == "h":
                            nc.vector.tensor_copy(ot[:, pad_lo:pad_lo + tw],
                                                  y32[:, :])
                            oh = ot
                        else:
                            nc.vector.tensor_sub(ot[:, pad_lo:pad_lo + tw],
                                                 y32[:, :],
                                                 oh[:, pad_lo:pad_lo + tw])
                        nc.sync.dma_start(
                            out=act[li, b, co, p][:, out_lo:out_lo + w_out],
                            in_=ot[:, :])

            def conv_layer(li, K, tiles_spec, get_in, wt, n_cin, psump,
                           conv_in, conv_out, out_kind):
                cin_p = wt["h", 0, 0].shape[0]
                for b in range(B_LOC):
                    for (t0, tw, in_lo, in_w, out_lo, pad_lo, pad_hi,
                         stride) in tiles_spec:
                        ins = {}
                        for ci in range(n_cin):
                            for p in ("h", "l"):
                                it = conv_in.tile([cin_p, in_w], FP16,
                                                  tag=f"in{ci}{p}", bufs=2,
                                                  name=f"cin{li}_{ci}{p}")
                                nc.sync.dma_start(
                                    out=it, in_=get_in(b, ci, p, in_lo, in_w))
                                ins[ci, p] = it
                        for co in range(NCH):
                            ps = psump.tile([128, tw], F32, tag="cps")
                            nmm = n_cin * K * 3
                            i = 0
                            for ci in range(n_cin):
                                for k in range(K):
                                    if stride == 1:
                                        sl = slice(k, k + tw)
                                    else:
                                        sl = slice(k, k + 2 * (tw - 1) + 1, 2)
                                    wh = wt["h", k, ci][:, co * 128:(co + 1) * 128]
                                    wl = wt["l", k, ci][:, co * 128:(co + 1) * 128]
                                    for lhsT, rhs in ((wh, ins[ci, "h"][:, sl]),
                                                      (wh, ins[ci, "l"][:, sl]),
                                                      (wl, ins[ci, "h"][:, sl])):
                                        nc.tensor.matmul(ps[:, :], lhsT, rhs,
                                                         start=(i == 0),
                                                         stop=(i == nmm - 1))
                                        i += 1

                            def write(dest, ps=ps, li=li, co=co):
                                nc.scalar.activation(
                                    out=dest, in_=ps[:, :], func=RELU,
                                    bias=bn_b[li - 1, co][:, :],
                                    scale=bn_s[li - 1, co][:, :])
                            emit_out(li, b, co, write, tw, out_lo, pad_lo,
                                     pad_hi, out_kind, conv_out)

            # D-transform index specs for F(2,3): (in0_off, in1_off, op)
            DSPEC = [(0, 2, "sub"), (1, 2, "add"), (2, 1, "sub"), (1, 3, "sub")]

            def conv_layer_wino(li, tiles_spec, get_in32, wt, psump,
                                conv_in, conv_out, out_kind):
                J = 256
                for b in range(B_LOC):
                    for (t0, tw, in_lo, in_w, out_lo, pad_lo, pad_hi) in tiles_spec:
                        dh = {}
                        dl = {}
                        for ci in range(NCH):
                            x32 = conv_in.tile([128, in_w], F32, tag=f"x{ci}",
                                               bufs=2, name=f"x{li}_{ci}")
                            nc.sync.dma_start(out=x32,
                                              in_=get_in32(b, ci, in_lo, in_w))
                            for i, (a0, a1, op) in enumerate(DSPEC):
                                s0 = x32[:, slice(a0, a0 + 2 * (J - 1) + 1, 2)]
                                s1 = x32[:, slice(a1, a1 + 2 * (J - 1) + 1, 2)]
                                d32 = conv_in.tile([128, J], F32, tag="d32",
                                                   bufs=4, name=f"d32_{li}")
                                if op == "sub":
                                    nc.vector.tensor_sub(d32[:, :], s0, s1)
                                else:
                                    nc.vector.tensor_add(d32[:, :], s0, s1)
                                h = conv_in.tile([128, J], FP16,
                                                 tag=f"dh{ci}_{i}", bufs=2,
                                                 name=f"dh{li}_{ci}_{i}")
                                nc.scalar.activation(out=h[:, :], in_=d32[:, :],
                                                     func=COPY)
                                dsub = conv_in.tile([128, J], F32, tag="dsub",
                                                    bufs=4, name=f"dsub_{li}")
                                nc.vector.tensor_sub(dsub[:, :], d32[:, :],
                                                     h[:, :])
                                l = conv_in.tile([128, J], FP16,
                                                 tag=f"dl{ci}_{i}", bufs=2,
                                                 name=f"dl{li}_{ci}_{i}")
                                nc.vector.tensor_copy(l[:, :], dsub[:, :])
                                dh[ci, i] = h
                                dl[ci, i] = l
                        for co in range(NCH):
                            Mp = psump.tile([128, 4, J], F32, tag="M")
                            for i in range(4):
                                t = 0
                                for ci in range(NCH):
                                    gh = wt["h", i, ci][:, co * 128:(co + 1) * 128]
                                    gl = wt["l", i, ci][:, co * 128:(co + 1) * 128]
                                    for lhsT, rhs in ((gh, dh[ci, i]),
                                                      (gh, dl[ci, i]),
                                                      (gl, dh[ci, i])):
                                        nc.tensor.matmul(
                                            Mp[:, i, :], lhsT, rhs[:, :],
                                            start=(t == 0),
                                            stop=(t == NCH * 3 - 1))
                                        t += 1
                            # assembly: ye = m0+m1+m2 ; yo = m1-m2-m3
                            m1s = conv_out.tile([128, J], F32, tag="m1s",
                                                bufs=2, name=f"m1s{li}")
                            nc.scalar.activation(out=m1s[:, :],
                                                 in_=Mp[:, 1, :], func=COPY)
                            m2s = conv_out.tile([128, J], F32, tag="m2s",
                                                bufs=2, name=f"m2s{li}")
                            nc.scalar.activation(out=m2s[:, :],
                                                 in_=Mp[:, 2, :], func=COPY)
                            ye1 = conv_out.tile([128, J], F32, tag="ye1",
                                                bufs=2, name=f"ye1{li}")
                            nc.vector.tensor_add(ye1[:, :], Mp[:, 0, :],
                                                 m1s[:, :])
                            ye2 = conv_out.tile([128, J], F32, tag="ye2",
                                                bufs=2, name=f"ye2{li}")
                            nc.vector.tensor_add(ye2[:, :], ye1[:, :],
                                                 m2s[:, :])
                            yo1 = conv_out.tile([128, J], F32, tag="yo1",
                                                bufs=2, name=f"yo1{li}")
                            nc.vector.tensor_sub(yo1[:, :], m1s[:, :],
                                                 m2s[:, :])
                            yo2 = conv_out.tile([128, J], F32, tag="yo2",
                                                bufs=2, name=f"yo2{li}")
                            nc.vector.tensor_sub(yo2[:, :], yo1[:, :],
                                                 Mp[:, 3, :])

                            def write(dest, li=li, co=co, ye2=ye2, yo2=yo2,
                                      tw=tw):
                                nc.scalar.activation(
                                    out=dest[:, slice(0, tw, 2)],
                                    in_=ye2[:, 0:(tw + 1) // 2], func=RELU,
                                    bias=bn_b[li - 1, co][:, :],
                                    scale=bn_s[li - 1, co][:, :])
                                nc.scalar.activation(
                                    out=dest[:, slice(1, tw, 2)],
                                    in_=yo2[:, 0:tw // 2], func=RELU,
                                    bias=bn_b[li - 1, co][:, :],
                                    scale=bn_s[li - 1, co][:, :])
                            emit_out(li, b, co, write, tw, out_lo, pad_lo,
                                     pad_hi, out_kind, conv_out)

            # ---- L1: direct conv(80->768, K=3, valid) -> act1 fp32 ---------
            with tc.tile_pool(name="w1p", bufs=1) as wp, \
                 tc.tile_pool(name="c1i", bufs=1) as conv_in, \
                 tc.tile_pool(name="c1o", bufs=1) as conv_out, \
                 tc.tile_pool(name="ps1", bufs=2, space="PSUM") as psump:
                wt = load_weights(wp, 1, 3, CIN, 1)
                emit_const_dmas()
                spec = [(0, 511, 0, 513, 0, 1, 0, 1),
                        (511, 511, 511, 513, 512, 0, 3, 1)]
                mels_d = {"h": mels_h, "l": mels_l}
                conv_layer(1, 3, spec,
                           lambda b, ci, p, lo, w: mels_d[p][b, :, lo:lo + w],
                           wt, 1, psump, conv_in, conv_out, "f")

            # ---- L2: Winograd conv(768->768) -> act2 h/l -------------------
            with tc.tile_pool(name="w2p", bufs=1) as wp, \
                 tc.tile_pool(name="c2i", bufs=1) as conv_in, \
                 tc.tile_pool(name="c2o", bufs=1) as conv_out, \
                 tc.tile_pool(name="ps2", bufs=2, space="PSUM") as psump:
                wt = load_weights(wp, 2, 4, 128, NCH)
                spec = [(0, 511, 0, 514, 0, 1, 0), (511, 511, 511, 514, 512, 0, 1)]
                conv_layer_wino(2, spec,
                                lambda b, ci, lo, w: act[1, b, ci, "f"][:, lo:lo + w],
                                wt, psump, conv_in, conv_out, "hl")

            # ---- L3: direct strided conv -> act3 fp32 ----------------------
            with tc.tile_pool(name="w3p", bufs=1) as wp, \
                 tc.tile_pool(name="c3i", bufs=1) as conv_in, \
                 tc.tile_pool(name="c3o", bufs=1) as conv_out, \
                 tc.tile_pool(name="ps3", bufs=2, space="PSUM") as psump:
                wt = load_weights(wp, 3, 4, 128, NCH)
                spec = [(0, 511, 0, 1024, 0, 1, 4, 2)]
                conv_layer(3, 4, spec,
                           lambda b, ci, p, lo, w: act[2, b, ci, p][:, lo:lo + w],
                           wt, NCH, psump, conv_in, conv_out, "f")

            # ---- L4: Winograd -> act4 fp32 ---------------------------------
            with tc.tile_pool(name="w4p", bufs=1) as wp, \
                 tc.tile_pool(name="c4i", bufs=1) as conv_in, \
                 tc.tile_pool(name="c4o", bufs=1) as conv_out, \
                 tc.tile_pool(name="ps4", bufs=2, space="PSUM") as psump:
                wt = load_weights(wp, 4, 4, 128, NCH)
                spec = [(0, 511, 0, 514, 0, 1, 4)]
                conv_layer_wino(4, spec,
                                lambda b, ci, lo, w: act[3, b, ci, "f"][:, lo:lo + w],
                                wt, psump, conv_in, conv_out, "f")

            # ---- L5: Winograd -> act5 h/l ----------------------------------
            with tc.tile_pool(name="w5p", bufs=1) as wp, \
                 tc.tile_pool(name="c5i", bufs=1) as conv_in, \
                 tc.tile_pool(name="c5o", bufs=1) as conv_out, \
                 tc.tile_pool(name="ps5", bufs=2, space="PSUM") as psump:
                wt = load_weights(wp, 5, 4, 128, NCH)
                spec = [(0, 511, 0, 514, 0, 0, 0)]
                conv_layer_wino(5, spec,
                                lambda b, ci, lo, w: act[4, b, ci, "f"][:, lo:lo + w],
                                wt, psump, conv_in, conv_out, "hl")

            # ---- L6 (1x1 conv + bias) and VQ ------------------------------
            with tc.tile_pool(name="w6p", bufs=1) as wp, \
                 tc.tile_pool(name="c6i", bufs=1) as conv_in, \
                 tc.tile_pool(name="vq", bufs=2) as vqp, \
                 tc.tile_pool(name="vqsmall", bufs=4) as vqsp, \
                 tc.tile_pool(name="ps6", bufs=2, space="PSUM") as psump:
                wt6 = {}
                for p in ("h", "l"):
                    for ci in range(NCH):
                        t = wp.tile([128, D], FP16, tag=f"w6{p}_{ci}",
                                    name=f"w6{p}_{ci}")
                        nc.sync.dma_start(
                            out=t, in_=wts[6, p][ci * 128:(ci + 1) * 128, :])
                        wt6[p, ci] = t
                for b in range(B_LOC):
                    ins = {}
                    for ci in range(NCH):
                        for p in ("h", "l"):
                            it = conv_in.tile([128, T2], FP16, tag=f"in{ci}{p}",
                                              bufs=2, name=f"cin6_{ci}{p}")
                            nc.sync.dma_start(out=it, in_=act[5, b, ci, p][:, :])
                            ins[ci, p] = it
                    ps = psump.tile([D, T2], F32, tag="zps")
                    nmm = NCH * 3
                    i = 0
                    for ci in range(NCH):
                        for lhsT, rhs in ((wt6["h", ci], ins[ci, "h"]),
                                          (wt6["h", ci], ins[ci, "l"]),
                                          (wt6["l", ci], ins[ci, "h"])):
                            nc.tensor.matmul(ps[:, :], lhsT[:, :], rhs[:, :],
                                             start=(i == 0), stop=(i == nmm - 1))
                            i += 1
                    zb = vqp.tile([D, T2], F32, tag="zb")
                    nc.vector.tensor_scalar_add(zb[:, :], ps[:, :], b6s[:, :])
                    nc.sync.dma_start(out=z_out[b, :, :], in_=zb[:, :])

                    zsq = vqp.tile([D, T2], F32, tag="zsq")
                    nc.vector.tensor_mul(zsq[:, :], zb[:, :], zb[:, :])
                    for c in range(4):
                        c0 = c * 128
                        cs = min(128, T2 - c0)
                        x2p = psump.tile([128, 1], F32, tag="x2p")
                        nc.tensor.matmul(x2p[:cs, :], zsq[:, c0:c0 + cs],
                                         ones64[:, :], start=True, stop=True)
                        x2s = vqsp.tile([128, 1], F32, tag="x2s")
                        nc.vector.tensor_copy(x2s[:cs, :], x2p[:cs, :])
                        scp = psump.tile([128, M], F32, tag="scp")
                        nc.tensor.matmul(scp[:cs, :], zb[:, c0:c0 + cs],
                                         embT2s[:, :], start=True, stop=True)
                        t1 = vqsp.tile([128, M], F32, tag="t1")
                        nc.vector.tensor_scalar_sub(t1[:cs, :], e2ns[:cs, :],
                                                    x2s[:cs, :])
                        nd = vqsp.tile([128, M], F32, tag="nd")
                        nc.vector.tensor_add(nd[:cs, :], t1[:cs, :], scp[:cs, :])
                        mx = vqsp.tile([128, 8], F32, tag="mx")
                        nc.vector.max(mx[:cs, :], nd[:cs, :])
                        mi = vqsp.tile([128, 8], U32, tag="mi")
                        nc.vector.max_index(mi[:cs, :], mx[:cs, :], nd[:cs, :])
                        col = b * 4 + c
                        nc.vector.tensor_copy(idxacc[:cs, col:col + 1],
                                              mi[:cs, 0:1])
            nc.sync.dma_start(out=idx_out[:, :], in_=idxacc[:, :])

    nc.compile()
    return nc


def _get_nc():
    global _CACHED_NC
    if _CACHED_NC is None:
        _CACHED_NC = _build_nc()
    return _CACHED_NC


def _split_hl(x):
    h = x.astype(np.float16)
    l = (x - h.astype(np.float32)).astype(np.float16)
    return np.ascontiguousarray(h), np.ascontiguousarray(l)


def _host_prep(inputs):
    f = np.float32
    out = {}
    for li, key in ((1, "w1"), (3, "w3")):
        wT = np.ascontiguousarray(inputs[key].astype(f).transpose(2, 1, 0))
        out[f"w{li}Th"], out[f"w{li}Tl"] = _split_hl(wT)
    for li, key in ((2, "w2"), (4, "w4"), (5, "w5")):
        wT = inputs[key].astype(f).transpose(2, 1, 0)   # [3, Cin, Cout]
        g = np.empty((4,) + wT.shape[1:], f)
        g[0] = wT[0]
        g[1] = (wT[0] + wT[1] + wT[2]) * f(0.5)
        g[2] = (wT[0] - wT[1] + wT[2]) * f(0.5)
        g[3] = wT[2]
        out[f"w{li}Gh"], out[f"w{li}Gl"] = _split_hl(g)
    w6 = np.ascontiguousarray(inputs["w6"].astype(f)[:, :, 0].T)
    out["w6Th"], out["w6Tl"] = _split_hl(w6)
    gamma = inputs["bn_gamma"].astype(f)
    beta = inputs["bn_beta"].astype(f)
    mean = inputs["bn_mean"].astype(f)
    var = inputs["bn_var"].astype(f)
    inv = gamma / np.sqrt(var + f(1e-5))
    bias = beta - mean * inv
    out["bnS"] = np.ascontiguousarray(inv.reshape(5, NCH, 128, 1))
    out["bnB"] = np.ascontiguousarray(bias.reshape(5, NCH, 128, 1))
    out["b6v"] = np.ascontiguousarray(inputs["b6"].astype(f).reshape(D, 1))
    emb = inputs["embedding"].astype(f)
    out["embT2"] = np.ascontiguousarray(2.0 * emb.T)
    e2 = np.sum(emb.astype(np.float64) ** 2, axis=1).astype(f)
    out["e2n"] = np.ascontiguousarray(np.broadcast_to(-e2[None, :], (128, M)))
    return out, emb


def _make_in_maps(inputs):
    shared, emb = _host_prep(inputs)
    mels = inputs["mels"].astype(np.float32)
    B = mels.shape[0]
    assert B == N_CORES * B_LOC
    in_maps = []
    for c in range(N_CORES):
        m = dict(shared)
        mh, ml = _split_hl(mels[c * B_LOC:(c + 1) * B_LOC])
        m["mels_h"] = mh
        m["mels_l"] = ml
        in_maps.append(m)
    return in_maps, emb


def kernel(**inputs):
    nc = _get_nc()
    in_maps, emb = _make_in_maps(inputs)
    B = N_CORES * B_LOC

    res = run_bass_kernel_spmd(nc, in_maps, core_ids=list(range(N_CORES)))

    z_parts = []
    idx_parts = []
    for c in range(N_CORES):
        r = res.results[c]
        z_parts.append(r["z_out"])
        arr = r["idx_out"]
        loc = np.empty(B_LOC * T2, dtype=np.int64)
        for b in range(B_LOC):
            for ch in range(4):
                c0 = ch * 128
                cs = min(128, T2 - c0)
                loc[b * T2 + c0: b * T2 + c0 + cs] = arr[:cs, b * 4 + ch]
        idx_parts.append(loc)

    z = np.concatenate(z_parts, axis=0).transpose(0, 2, 1)
    z = np.ascontiguousarray(z)
    idx = np.concatenate(idx_parts)

    q = emb[idx].reshape(B, T2, D)
    q_st = z + (q - z)
    diff = z.astype(np.float64) - q.astype(np.float64)
    loss = np.float32(0.25 * np.mean(diff * diff))
    counts = np.bincount(idx, minlength=M).astype(np.float64)
    avg = counts / idx.shape[0]
    perplexity = np.float32(np.exp(-np.sum(avg * np.log(avg + 1e-10))))
    return q_st, loss, perplexity


# revision 16
# speedup vs baseline: 1.0177x; 1.0029x over previous
"""Trainium2 Bass kernel for nn_Encoder (conv stack + VQ codebook).

Reference computation (fp32):
  x = mels [32, 80, 1024]
  5x (conv1d + batchnorm-affine + relu), 1x 1x1-conv + bias  -> z [32, 64, 511]
  VQ: nearest codebook row (squared L2, 512 codes, D=64) -> q_st, loss, perplexity

Sharding: data-parallel over batch across 8 NeuronCores (4 batches/core);
conv weights + codebook replicated.

Speed tricks (all verified to keep VQ argmins identical to the fp32 ref):
- fp16 hi/lo 3-term matmuls (W=Wh+Wl, X=Xh+Xl; Y ~= Wh.Xh + Wh.Xl + Wl.Xh):
  1 PE cycle/row instead of fp32's 4, output rel err ~3e-6 (the PE supports
  fp16 subnormals, so the lo parts need no scaling).
- Winograd F(2,3) on the stride-1 K=3 768x768 layers (L2/L4/L5): 4 transformed
  256-col matmuls per output pair instead of 3x511 direct columns (1.5x fewer
  PE cycles), transforms on DVE/ACT in fp32.
- The VQ distance computation stays fully fp32 and mirrors the reference's
  operation order; host finishes gather/loss/perplexity exactly as the
  reference does.
"""

import numpy as np

import concourse.bass as bass
import concourse.tile as tile
from concourse import bacc, mybir
from concourse.bass_utils import run_bass_kernel_spmd

N_CORES = 8
B_LOC = 4
CIN = 80
C = 768
D = 64
M = 512
T0 = 1024
T2 = 511
NCH = C // 128
F32 = mybir.dt.float32
FP16 = mybir.dt.float16
U32 = mybir.dt.uint32
RELU = mybir.ActivationFunctionType.Relu
COPY = mybir.ActivationFunctionType.Copy

_CACHED_NC = None


def _build_nc():
    nc = bacc.Bacc("TRN2", target_bir_lowering=False, debug=False,
                   num_devices=N_CORES)

    mels_h = nc.dram_tensor("mels_h", [B_LOC, CIN, T0], FP16, kind="ExternalInput")
    mels_l = nc.dram_tensor("mels_l", [B_LOC, CIN, T0], FP16, kind="ExternalInput")
    wts = {}
    for li, K, cin in ((1, 3, CIN), (3, 4, C)):
        for p in ("h", "l"):
            wts[li, p] = nc.dram_tensor(f"w{li}T{p}", [K, cin, C], FP16,
                                        kind="ExternalInput")
    for li in (2, 4, 5):  # Winograd G-transformed weights
        for p in ("h", "l"):
            wts[li, p] = nc.dram_tensor(f"w{li}G{p}", [4, C, C], FP16,
                                        kind="ExternalInput")
    for p in ("h", "l"):
        wts[6, p] = nc.dram_tensor(f"w6T{p}", [C, D], FP16, kind="ExternalInput")
    bnS = nc.dram_tensor("bnS", [5, NCH, 128, 1], F32, kind="ExternalInput")
    bnB = nc.dram_tensor("bnB", [5, NCH, 128, 1], F32, kind="ExternalInput")
    b6v = nc.dram_tensor("b6v", [D, 1], F32, kind="ExternalInput")
    embT2 = nc.dram_tensor("embT2", [D, M], F32, kind="ExternalInput")
    e2n = nc.dram_tensor("e2n", [128, M], F32, kind="ExternalInput")

    z_out = nc.dram_tensor("z_out", [B_LOC, D, T2], F32, kind="ExternalOutput")
    idx_out = nc.dram_tensor("idx_out", [128, 16], U32, kind="ExternalOutput")

    with tile.TileContext(nc) as tc:
        with (
            tc.tile_pool(name="const", bufs=1) as constp,
            tc.tile_pool(name="dram", bufs=1, space="DRAM") as dramp,
        ):
            # Const tiles are allocated here, but their (slow, strided) DMAs
            # are emitted after L1's weight loads, on the gpsimd SWDGE queues,
            # so they don't delay the first conv matmuls.
            bn_s = {}
            bn_b = {}
            for li in range(5):
                for ci in range(NCH):
                    s = constp.tile([128, 1], F32, tag=f"bns_{li}_{ci}")
                    b = constp.tile([128, 1], F32, tag=f"bnb_{li}_{ci}")
                    bn_s[li, ci] = s
                    bn_b[li, ci] = b
            b6s = constp.tile([D, 1], F32, tag="b6s")
            embT2s = constp.tile([D, M], F32, tag="embT2s")
            e2ns = constp.tile([128, M], F32, tag="e2ns")
            ones64 = constp.tile([D, 1], F32, tag="ones64")
            nc.vector.memset(ones64, 1.0)
            idxacc = constp.tile([128, 16], U32, tag="idxacc")

            def emit_bn0_dmas():
                # L1's ReLU scale/bias must arrive fast or the first psum
                # drains block and the PE never warms (HAM stays at K=4/8).
                for ci in range(NCH):
                    nc.sync.dma_start(out=bn_s[0, ci], in_=bnS[0, ci, :, :])
                    nc.sync.dma_start(out=bn_b[0, ci], in_=bnB[0, ci, :, :])

            def emit_const_dmas():
                for li in range(1, 5):
                    for ci in range(NCH):
                        nc.gpsimd.dma_start(out=bn_s[li, ci],
                                            in_=bnS[li, ci, :, :])
                        nc.gpsimd.dma_start(out=bn_b[li, ci],
                                            in_=bnB[li, ci, :, :])
                nc.gpsimd.dma_start(out=b6s, in_=b6v[:, :])
                nc.gpsimd.dma_start(out=embT2s, in_=embT2[:, :])
                nc.gpsimd.dma_start(out=e2ns, in_=e2n[:, :])

            # DRAM scratch. fp32 acts feed Winograd layers; fp16 h/l pairs
            # feed the direct layers (L3, L6).
            act = {}
            for li, width, kind in ((1, 1026, "f"), (2, T0, "hl"),
                                    (3, 516, "f"), (4, 516, "f"),
                                    (5, T2, "hl")):
                for b in range(B_LOC):
                    for ci in range(NCH):
                        if kind == "f":
                            act[li, b, ci, "f"] = dramp.tile(
                                [128, width], F32, tag=f"a{li}_{b}_{ci}",
                                name=f"a{li}_{b}_{ci}")
                        else:
                            for p in ("h", "l"):
                                act[li, b, ci, p] = dramp.tile(
                                    [128, width], FP16, tag=f"a{li}_{b}_{ci}{p}",
                                    name=f"a{li}_{b}_{ci}{p}")

            def load_weights(wp, li, K, cin_p, n_cin):
                wt = {}
                for p in ("h", "l"):
                    for k in range(K):
                        for ci in range(n_cin):
                            t = wp.tile([cin_p, C], FP16, tag=f"w{li}{p}_{k}_{ci}",
                                        name=f"w{li}{p}_{k}_{ci}")
                            nc.sync.dma_start(
                                out=t,
                                in_=wts[li, p][k, ci * cin_p:(ci + 1) * cin_p, :])
                            wt[p, k, ci] = t
                return wt

            def emit_out(li, b, co, src_ap_writer, tw, out_lo, pad_lo, pad_hi,
                         out_kind, conv_out):
                """src_ap_writer(dest_ap): emits the ACT op(s) writing the
                activated output rows into dest_ap (width tw)."""
                w_out = pad_lo + tw + pad_hi
                if out_kind == "f":
                    ot = conv_out.tile([128, w_out], F32, tag="of", bufs=3,
                                       name=f"of{li}_{co}")
                    if pad_lo:
                        nc.vector.memset(ot[:, 0:pad_lo], 0.0)
                    if pad_hi:
                        nc.vector.memset(ot[:, pad_lo + tw:w_out], 0.0)
                    src_ap_writer(ot[:, pad_lo:pad_lo + tw])
                    nc.sync.dma_start(
                        out=act[li, b, co, "f"][:, out_lo:out_lo + w_out],
                        in_=ot[:, :])
                else:
                    y32 = conv_out.tile([128, tw], F32, tag="y", bufs=3,
                                        name=f"y{li}_{co}")
                    src_ap_writer(y32[:, :])
                    oh = None
                    for p in ("h", "l"):
                        ot = conv_out.tile([128, w_out], FP16, tag=f"o{p}",
                                           bufs=4, name=f"o{li}_{co}{p}")
                        if pad_lo:
                            nc.vector.memset(ot[:, 0:pad_lo], 0.0)
                        if pad_hi:
                            nc.vector.memset(ot[:, pad_lo + tw:w_out], 0.0)
                        if p == "h":
                            nc.vector.tensor_copy(ot[:, pad_lo:pad_lo + tw],
                                                  y32[:, :])
                            oh = ot
                        else:
                            nc.vector.tensor_sub(ot[:, pad_lo:pad_lo + tw],
                                                 y32[:, :],
                                                 oh[:, pad_lo:pad_lo + tw])
                        nc.sync.dma_start(
                            out=act[li, b, co, p][:, out_lo:out_lo + w_out],
                            in_=ot[:, :])

            def conv_layer(li, K, tiles_spec, get_in, wt, n_cin, psump,
                           conv_in, conv_out, out_kind):
                cin_p = wt["h", 0, 0].shape[0]
                for b in range(B_LOC):
                    for (t0, tw, in_lo, in_w, out_lo, pad_lo, pad_hi,
                         stride) in tiles_spec:
                        ins = {}
                        for ci in range(n_cin):
                            for p in ("h", "l"):
                                it = conv_in.tile([cin_p, in_w], FP16,
                                                  tag=f"in{ci}{p}", bufs=2,
                                                  name=f"cin{li}_{ci}{p}")
                                nc.sync.dma_start(
                                    out=it, in_=get_in(b, ci, p, in_lo, in_w))
                                ins[ci, p] = it
                        for co in range(NCH):
                            ps = psump.tile([128, tw], F32, tag="cps")
                            nmm = n_cin * K * 3
                            i = 0
                            for ci in range(n_cin):
                                for k in range(K):
                                    if stride == 1:
                                        sl = slice(k, k + tw)
                                    else:
                                        sl = slice(k, k + 2 * (tw - 1) + 1, 2)
                                    wh = wt["h", k, ci][:, co * 128:(co + 1) * 128]
                                    wl = wt["l", k, ci][:, co * 128:(co + 1) * 128]
                                    for lhsT, rhs in ((wh, ins[ci, "h"][:, sl]),
                                                      (wh, ins[ci, "l"][:, sl]),
                                                      (wl, ins[ci, "h"][:, sl])):
                                        nc.tensor.matmul(ps[:, :], lhsT, rhs,
                                                         start=(i == 0),
                                                         stop=(i == nmm - 1))
                                        i += 1

                            def write(dest, ps=ps, li=li, co=co):
                                nc.scalar.activation(
                                    out=dest, in_=ps[:, :], func=RELU,
                                    bias=bn_b[li - 1, co][:, :],
                                    scale=bn_s[li - 1, co][:, :])
                            emit_out(li, b, co, write, tw, out_lo, pad_lo,
                                     pad_hi, out_kind, conv_out)

            # D-transform index specs for F(2,3): (in0_off, in1_off, op)
            DSPEC = [(0, 2, "sub"), (1, 2, "add"), (2, 1, "sub"), (1, 3, "sub")]

            def conv_layer_wino(li, tiles_spec, get_in32, wt, psump,
                                conv_in, conv_out, out_kind):
                J = 256
                for b in range(B_LOC):
                    for (t0, tw, in_lo, in_w, out_lo, pad_lo, pad_hi) in tiles_spec:
                        dh = {}
                        dl = {}
                        for ci in range(NCH):
                            x32 = conv_in.tile([128, in_w], F32, tag=f"x{ci}",
                                               bufs=2, name=f"x{li}_{ci}")
                            nc.sync.dma_start(out=x32,
                                              in_=get_in32(b, ci, in_lo, in_w))
                            for i, (a0, a1, op) in enumerate(DSPEC):
                                s0 = x32[:, slice(a0, a0 + 2 * (J - 1) + 1, 2)]
                                s1 = x32[:, slice(a1, a1 + 2 * (J - 1) + 1, 2)]
                                d32 = conv_in.tile([128, J], F32, tag="d32",
                                                   bufs=4, name=f"d32_{li}")
                                if op == "sub":
                                    nc.vector.tensor_sub(d32[:, :], s0, s1)
                                else:
                                    nc.vector.tensor_add(d32[:, :], s0, s1)
                                h = conv_in.tile([128, J], FP16,
                                                 tag=f"dh{ci}_{i}", bufs=2,
                                                 name=f"dh{li}_{ci}_{i}")
                                nc.scalar.activation(out=h[:, :], in_=d32[:, :],
                                                     func=COPY)
                                dsub = conv_in.tile([128, J], F32, tag="dsub",
                                                    bufs=4, name=f"dsub_{li}")
                                nc.vector.tensor_sub(dsub[:, :], d32[:, :],
                                                     h[:, :])
                                l = conv_in.tile([128, J], FP16,
                                                 tag=f"dl{ci}_{i}", bufs=2,
                                                 name=f"dl{li}_{ci}_{i}")
                                nc.vector.tensor_copy(l[:, :], dsub[:, :])
                                dh[ci, i] = h
                                dl[ci, i] = l
                        for co in range(NCH):
                            Mp = psump.tile([128, 4, J], F32, tag="M")
                            for i in range(4):
                                t = 0
                                for ci in range(NCH):
                                    gh = wt["h", i, ci][:, co * 128:(co + 1) * 128]
                                    gl = wt["l", i, ci][:, co * 128:(co + 1) * 128]
                                    for lhsT, rhs in ((gh, dh[ci, i]),
                                                      (gh, dl[ci, i]),
                                                      (gl, dh[ci, i])):
                                        nc.tensor.matmul(
                                            Mp[:, i, :], lhsT, rhs[:, :],
                                            start=(t == 0),
                                            stop=(t == NCH * 3 - 1))
                                        t += 1
                            # assembly: ye = m0+m1+m2 ; yo = m1-m2-m3
                            m1s = conv_out.tile([128, J], F32, tag="m1s",
                                                bufs=2, name=f"m1s{li}")
                            nc.scalar.activation(out=m1s[:, :],
                                                 in_=Mp[:, 1, :], func=COPY)
                            m2s = conv_out.tile([128, J], F32, tag="m2s",
                                                bufs=2, name=f"m2s{li}")
                            nc.scalar.activation(out=m2s[:, :],
                                                 in_=Mp[:, 2, :], func=COPY)
                            ye1 = conv_out.tile([128, J], F32, tag="ye1",
                                                bufs=2, name=f"ye1{li}")
                            nc.vector.tensor_add(ye1[:, :], Mp[:, 0, :],
                                                 m1s[:, :])
                            ye2 = conv_out.tile([128, J], F32, tag="ye2",
                                                bufs=2, name=f"ye2{li}")
                            nc.vector.tensor_add(ye2[:, :], ye1[:, :],
                                                 m2s[:, :])
                            yo1 = conv_out.tile([128, J], F32, tag="yo1",
                                                bufs=2, name=f"yo1{li}")
                            nc.vector.tensor_sub(yo1[:, :], m1s[:, :],
                                                 m2s[:, :])
                            yo2 = conv_out.tile([128, J], F32, tag="yo2",
                                                bufs=2, name=f"yo2{li}")
                            nc.vector.tensor_sub(yo2[:, :], yo1[:, :],
                                                 Mp[:, 3, :])

                            def write(dest, li=li, co=co, ye2=ye2, yo2=yo2,
                                      tw=tw):
                                nc.scalar.activation(
                                    out=dest[:, slice(0, tw, 2)],
                                    in_=ye2[:, 0:(tw + 1) // 2], func=RELU,
                                    bias=bn_b[li - 1, co][:, :],
                                    scale=bn_s[li - 1, co][:, :])
                                nc.scalar.activation(
                                    out=dest[:, slice(1, tw, 2)],
                                    in_=yo2[:, 0:tw // 2], func=RELU,
                                    bias=bn_b[li - 1, co][:, :],
                                    scale=bn_s[li - 1, co][:, :])
                            emit_out(li, b, co, write, tw, out_lo, pad_lo,
                                     pad_hi, out_kind, conv_out)

            # ---- L1: direct conv(80->768, K=3, valid) -> act1 fp32 ---------
            with tc.tile_pool(name="w1p", bufs=1) as wp, \
                 tc.tile_pool(name="c1i", bufs=1) as conv_in, \
                 tc.tile_pool(name="c1o", bufs=1) as conv_out, \
                 tc.tile_pool(name="ps1", bufs=2, space="PSUM") as psump:
                wt = load_weights(wp, 1, 3, CIN, 1)
                emit_bn0_dmas()
                emit_const_dmas()
                spec = [(0, 511, 0, 513, 0, 1, 0, 1),
                        (511, 511, 511, 513, 512, 0, 3, 1)]
                mels_d = {"h": mels_h, "l": mels_l}
                conv_layer(1, 3, spec,
                           lambda b, ci, p, lo, w: mels_d[p][b, :, lo:lo + w],
                           wt, 1, psump, conv_in, conv_out, "f")

            # ---- L2: Winograd conv(768->768) -> act2 h/l -------------------
            with tc.tile_pool(name="w2p", bufs=1) as wp, \
                 tc.tile_pool(name="c2i", bufs=1) as conv_in, \
                 tc.tile_pool(name="c2o", bufs=1) as conv_out, \
                 tc.tile_pool(name="ps2", bufs=2, space="PSUM") as psump:
                wt = load_weights(wp, 2, 4, 128, NCH)
                spec = [(0, 511, 0, 514, 0, 1, 0), (511, 511, 511, 514, 512, 0, 1)]
                conv_layer_wino(2, spec,
                                lambda b, ci, lo, w: act[1, b, ci, "f"][:, lo:lo + w],
                                wt, psump, conv_in, conv_out, "hl")

            # ---- L3: direct strided conv -> act3 fp32 ----------------------
            with tc.tile_pool(name="w3p", bufs=1) as wp, \
                 tc.tile_pool(name="c3i", bufs=1) as conv_in, \
                 tc.tile_pool(name="c3o", bufs=1) as conv_out, \
                 tc.tile_pool(name="ps3", bufs=2, space="PSUM") as psump:
                wt = load_weights(wp, 3, 4, 128, NCH)
                spec = [(0, 511, 0, 1024, 0, 1, 4, 2)]
                conv_layer(3, 4, spec,
                           lambda b, ci, p, lo, w: act[2, b, ci, p][:, lo:lo + w],
                           wt, NCH, psump, conv_in, conv_out, "f")

            # ---- L4: Winograd -> act4 fp32 ---------------------------------
            with tc.tile_pool(name="w4p", bufs=1) as wp, \
                 tc.tile_pool(name="c4i", bufs=1) as conv_in, \
                 tc.tile_pool(name="c4o", bufs=1) as conv_out, \
                 tc.tile_pool(name="ps4", bufs=2, space="PSUM") as psump:
                wt = load_weights(wp, 4, 4, 128, NCH)
                spec = [(0, 511, 0, 514, 0, 1, 4)]
                conv_layer_wino(4, spec,
                                lambda b, ci, lo, w: act[3, b, ci, "f"][:, lo:lo + w],
                                wt, psump, conv_in, conv_out, "f")

            # ---- L5: Winograd -> act5 h/l ----------------------------------
            with tc.tile_pool(name="w5p", bufs=1) as wp, \
                 tc.tile_pool(name="c5i", bufs=1) as conv_in, \
                 tc.tile_pool(name="c5o", bufs=1) as conv_out, \
                 tc.tile_pool(name="ps5", bufs=2, space="PSUM") as psump:
                wt = load_weights(wp, 5, 4, 128, NCH)
                spec = [(0, 511, 0, 514, 0, 0, 0)]
                conv_layer_wino(5, spec,
                                lambda b, ci, lo, w: act[4, b, ci, "f"][:, lo:lo + w],
                                wt, psump, conv_in, conv_out, "hl")

            # ---- L6 (1x1 conv + bias) and VQ ------------------------------
            with tc.tile_pool(name="w6p", bufs=1) as wp, \
                 tc.tile_pool(name="c6i", bufs=1) as conv_in, \
                 tc.tile_pool(name="vq", bufs=2) as vqp, \
                 tc.tile_pool(name="vqsmall", bufs=4) as vqsp, \
                 tc.tile_pool(name="ps6", bufs=2, space="PSUM") as psump:
                wt6 = {}
                for p in ("h", "l"):
                    for ci in range(NCH):
                        t = wp.tile([128, D], FP16, tag=f"w6{p}_{ci}",
                                    name=f"w6{p}_{ci}")
                        nc.sync.dma_start(
                            out=t, in_=wts[6, p][ci * 128:(ci + 1) * 128, :])
                        wt6[p, ci] = t
                for b in range(B_LOC):
                    ins = {}
                    for ci in range(NCH):
                        for p in ("h", "l"):
                            it = conv_in.tile([128, T2], FP16, tag=f"in{ci}{p}",
                                              bufs=2, name=f"cin6_{ci}{p}")
                            nc.sync.dma_start(out=it, in_=act[5, b, ci, p][:, :])
                            ins[ci, p] = it
                    ps = psump.tile([D, T2], F32, tag="zps")
                    nmm = NCH * 3
                    i = 0
                    for ci in range(NCH):
                        for lhsT, rhs in ((wt6["h", ci], ins[ci, "h"]),
                                          (wt6["h", ci], ins[ci, "l"]),
                                          (wt6["l", ci], ins[ci, "h"])):
                            nc.tensor.matmul(ps[:, :], lhsT[:, :], rhs[:, :],
                                             start=(i == 0), stop=(i == nmm - 1))
                            i += 1
                    zb = vqp.tile([D, T2], F32, tag="zb")
                    nc.vector.tensor_scalar_add(zb[:, :], ps[:, :], b6s[:, :])
                    nc.sync.dma_start(out=z_out[b, :, :], in_=zb[:, :])

                    zsq = vqp.tile([D, T2], F32, tag="zsq")
                    nc.vector.tensor_mul(zsq[:, :], zb[:, :], zb[:, :])
                    for c in range(4):
                        c0 = c * 128
                        cs = min(128, T2 - c0)
                        x2p = psump.tile([128, 1], F32, tag="x2p")
                        nc.tensor.matmul(x2p[:cs, :], zsq[:, c0:c0 + cs],
                                         ones64[:, :], start=True, stop=True)
                        x2s = vqsp.tile([128, 1], F32, tag="x2s")
                        nc.vector.tensor_copy(x2s[:cs, :], x2p[:cs, :])
                        scp = psump.tile([128, M], F32, tag="scp")
                        nc.tensor.matmul(scp[:cs, :], zb[:, c0:c0 + cs],
                                         embT2s[:, :], start=True, stop=True)
                        t1 = vqsp.tile([128, M], F32, tag="t1")
                        nc.vector.tensor_scalar_sub(t1[:cs, :], e2ns[:cs, :],
                                                    x2s[:cs, :])
                        nd = vqsp.tile([128, M], F32, tag="nd")
                        nc.vector.tensor_add(nd[:cs, :], t1[:cs, :], scp[:cs, :])
                        mx = vqsp.tile([128, 8], F32, tag="mx")
                        nc.vector.max(mx[:cs, :], nd[:cs, :])
                        mi = vqsp.tile([128, 8], U32, tag="mi")
                        nc.vector.max_index(mi[:cs, :], mx[:cs, :], nd[:cs, :])
                        col = b * 4 + c
                        nc.vector.tensor_copy(idxacc[:cs, col:col + 1],
                                              mi[:cs, 0:1])
            nc.sync.dma_start(out=idx_out[:, :], in_=idxacc[:, :])

    nc.compile()
    return nc


def _get_nc():
    global _CACHED_NC
    if _CACHED_NC is None:
        _CACHED_NC = _build_nc()
    return _CACHED_NC


def _split_hl(x):
    h = x.astype(np.float16)
    l = (x - h.astype(np.float32)).astype(np.float16)
    return np.ascontiguousarray(h), np.ascontiguousarray(l)


def _host_prep(inputs):
    f = np.float32
    out = {}
    for li, key in ((1, "w1"), (3, "w3")):
        wT = np.ascontiguousarray(inputs[key].astype(f).transpose(2, 1, 0))
        out[f"w{li}Th"], out[f"w{li}Tl"] = _split_hl(wT)
    for li, key in ((2, "w2"), (4, "w4"), (5, "w5")):
        wT = inputs[key].astype(f).transpose(2, 1, 0)   # [3, Cin, Cout]
        g = np.empty((4,) + wT.shape[1:], f)
        g[0] = wT[0]
        g[1] = (wT[0] + wT[1] + wT[2]) * f(0.5)
        g[2] = (wT[0] - wT[1] + wT[2]) * f(0.5)
        g[3] = wT[2]
        out[f"w{li}Gh"], out[f"w{li}Gl"] = _split_hl(g)
    w6 = np.ascontiguousarray(inputs["w6"].astype(f)[:, :, 0].T)
    out["w6Th"], out["w6Tl"] = _split_hl(w6)
    gamma = inputs["bn_gamma"].astype(f)
    beta = inputs["bn_beta"].astype(f)
    mean = inputs["bn_mean"].astype(f)
    var = inputs["bn_var"].astype(f)
    inv = gamma / np.sqrt(var + f(1e-5))
    bias = beta - mean * inv
    out["bnS"] = np.ascontiguousarray(inv.reshape(5, NCH, 128, 1))
    out["bnB"] = np.ascontiguousarray(bias.reshape(5, NCH, 128, 1))
    out["b6v"] = np.ascontiguousarray(inputs["b6"].astype(f).reshape(D, 1))
    emb = inputs["embedding"].astype(f)
    out["embT2"] = np.ascontiguousarray(2.0 * emb.T)
    e2 = np.sum(emb.astype(np.float64) ** 2, axis=1).astype(f)
    out["e2n"] = np.ascontiguousarray(np.broadcast_to(-e2[None, :], (128, M)))
    return out, emb


def _make_in_maps(inputs):
    shared, emb = _host_prep(inputs)
    mels = inputs["mels"].astype(np.float32)
    B = mels.shape[0]
    assert B == N_CORES * B_LOC
    in_maps = []
    for c in range(N_CORES):
        m = dict(shared)
        mh, ml = _split_hl(mels[c * B_LOC:(c + 1) * B_LOC])
        m["mels_h"] = mh
        m["mels_l"] = ml
        in_maps.append(m)
    return in_maps, emb


def kernel(**inputs):
    nc = _get_nc()
    in_maps, emb = _make_in_maps(inputs)
    B = N_CORES * B_LOC

    res = run_bass_kernel_spmd(nc, in_maps, core_ids=list(range(N_CORES)))

    z_parts = []
    idx_parts = []
    for c in range(N_CORES):
        r = res.results[c]
        z_parts.append(r["z_out"])
        arr = r["idx_out"]
        loc = np.empty(B_LOC * T2, dtype=np.int64)
        for b in range(B_LOC):
            for ch in range(4):
                c0 = ch * 128
                cs = min(128, T2 - c0)
                loc[b * T2 + c0: b * T2 + c0 + cs] = arr[:cs, b * 4 + ch]
        idx_parts.append(loc)

    z = np.concatenate(z_parts, axis=0).transpose(0, 2, 1)
    z = np.ascontiguousarray(z)
    idx = np.concatenate(idx_parts)

    q = emb[idx].reshape(B, T2, D)
    q_st = z + (q - z)
    diff = z.astype(np.float64) - q.astype(np.float64)
    loss = np.float32(0.25 * np.mean(diff * diff))
    counts = np.bincount(idx, minlength=M).astype(np.float64)
    avg = counts / idx.shape[0]
    perplexity = np.float32(np.exp(-np.sum(avg * np.log(avg + 1e-10))))
    return q_st, loss, perplexity


# revision 17
# speedup vs baseline: 1.0432x; 1.0251x over previous
"""Trainium2 Bass kernel for nn_Encoder (conv stack + VQ codebook).

Reference computation (fp32):
  x = mels [32, 80, 1024]
  5x (conv1d + batchnorm-affine + relu), 1x 1x1-conv + bias  -> z [32, 64, 511]
  VQ: nearest codebook row (squared L2, 512 codes, D=64) -> q_st, loss, perplexity

Sharding: data-parallel over batch across 8 NeuronCores (4 batches/core);
conv weights + codebook replicated.

Speed tricks (all verified to keep VQ argmins identical to the fp32 ref):
- fp16 hi/lo 3-term matmuls (W=Wh+Wl, X=Xh+Xl; Y ~= Wh.Xh + Wh.Xl + Wl.Xh):
  1 PE cycle/row instead of fp32's 4, output rel err ~3e-6 (the PE supports
  fp16 subnormals, so the lo parts need no scaling).
- Winograd F(2,3) on the stride-1 K=3 768x768 layers (L2/L4/L5): 4 transformed
  256-col matmuls per output pair instead of 3x511 direct columns (1.5x fewer
  PE cycles), transforms on DVE/ACT in fp32.
- The VQ distance computation stays fully fp32 and mirrors the reference's
  operation order; host finishes gather/loss/perplexity exactly as the
  reference does.
"""

import numpy as np

import concourse.bass as bass
import concourse.tile as tile
from concourse import bacc, mybir
from concourse.bass_utils import run_bass_kernel_spmd

N_CORES = 8
B_LOC = 4
CIN = 80
C = 768
D = 64
M = 512
T0 = 1024
T2 = 511
NCH = C // 128
F32 = mybir.dt.float32
FP16 = mybir.dt.float16
U32 = mybir.dt.uint32
RELU = mybir.ActivationFunctionType.Relu
COPY = mybir.ActivationFunctionType.Copy

_CACHED_NC = None


def _build_nc():
    nc = bacc.Bacc("TRN2", target_bir_lowering=False, debug=False,
                   num_devices=N_CORES)

    mels_h = nc.dram_tensor("mels_h", [B_LOC, CIN, T0], FP16, kind="ExternalInput")
    mels_l = nc.dram_tensor("mels_l", [B_LOC, CIN, T0], FP16, kind="ExternalInput")
    wts = {}
    for li, K, cin in ((1, 3, CIN), (3, 4, C)):
        for p in ("h", "l"):
            wts[li, p] = nc.dram_tensor(f"w{li}T{p}", [K, cin, C], FP16,
                                        kind="ExternalInput")
    for li in (2, 4, 5):  # Winograd G-transformed weights
        for p in ("h", "l"):
            wts[li, p] = nc.dram_tensor(f"w{li}G{p}", [4, C, C], FP16,
                                        kind="ExternalInput")
    for p in ("h", "l"):
        wts[6, p] = nc.dram_tensor(f"w6T{p}", [C, D], FP16, kind="ExternalInput")
    bnS = nc.dram_tensor("bnS", [5, NCH, 128, 1], F32, kind="ExternalInput")
    bnB = nc.dram_tensor("bnB", [5, NCH, 128, 1], F32, kind="ExternalInput")
    b6v = nc.dram_tensor("b6v", [D, 1], F32, kind="ExternalInput")
    embT2 = nc.dram_tensor("embT2", [D, M], F32, kind="ExternalInput")
    e2n = nc.dram_tensor("e2n", [128, M], F32, kind="ExternalInput")

    z_out = nc.dram_tensor("z_out", [B_LOC, D, T2], F32, kind="ExternalOutput")
    idx_out = nc.dram_tensor("idx_out", [128, 16], U32, kind="ExternalOutput")

    with tile.TileContext(nc) as tc:
        with (
            tc.tile_pool(name="const", bufs=1) as constp,
            tc.tile_pool(name="dram", bufs=1, space="DRAM") as dramp,
        ):
            # Const tiles are allocated here, but their (slow, strided) DMAs
            # are emitted after L1's weight loads, on the gpsimd SWDGE queues,
            # so they don't delay the first conv matmuls.
            bn_s = {}
            bn_b = {}
            for li in range(5):
                for ci in range(NCH):
                    s = constp.tile([128, 1], F32, tag=f"bns_{li}_{ci}")
                    b = constp.tile([128, 1], F32, tag=f"bnb_{li}_{ci}")
                    bn_s[li, ci] = s
                    bn_b[li, ci] = b
            b6s = constp.tile([D, 1], F32, tag="b6s")
            embT2s = constp.tile([D, M], F32, tag="embT2s")
            e2ns = constp.tile([128, M], F32, tag="e2ns")
            ones64 = constp.tile([D, 1], F32, tag="ones64")
            nc.vector.memset(ones64, 1.0)
            idxacc = constp.tile([128, 16], U32, tag="idxacc")

            def emit_bn0_dmas():
                # L1's ReLU scale/bias must arrive fast or the first psum
                # drains block and the PE never warms (HAM stays at K=4/8).
                for ci in range(NCH):
                    nc.sync.dma_start(out=bn_s[0, ci], in_=bnS[0, ci, :, :])
                    nc.sync.dma_start(out=bn_b[0, ci], in_=bnB[0, ci, :, :])

            def emit_const_dmas():
                for li in range(1, 5):
                    for ci in range(NCH):
                        nc.gpsimd.dma_start(out=bn_s[li, ci],
                                            in_=bnS[li, ci, :, :])
                        nc.gpsimd.dma_start(out=bn_b[li, ci],
                                            in_=bnB[li, ci, :, :])
                nc.gpsimd.dma_start(out=b6s, in_=b6v[:, :])
                nc.gpsimd.dma_start(out=embT2s, in_=embT2[:, :])
                nc.gpsimd.dma_start(out=e2ns, in_=e2n[:, :])

            # DRAM scratch. fp32 acts feed Winograd layers; fp16 h/l pairs
            # feed the direct layers (L3, L6).
            act = {}
            for li, width, kind in ((1, 1026, "f"), (2, T0, "hl"),
                                    (3, 516, "f"), (4, 516, "f"),
                                    (5, T2, "hl")):
                for b in range(B_LOC):
                    for ci in range(NCH):
                        if kind == "f":
                            act[li, b, ci, "f"] = dramp.tile(
                                [128, width], F32, tag=f"a{li}_{b}_{ci}",
                                name=f"a{li}_{b}_{ci}")
                        else:
                            for p in ("h", "l"):
                                act[li, b, ci, p] = dramp.tile(
                                    [128, width], FP16, tag=f"a{li}_{b}_{ci}{p}",
                                    name=f"a{li}_{b}_{ci}{p}")

            def load_weights(wp, li, K, cin_p, n_cin):
                wt = {}
                for p in ("h", "l"):
                    for k in range(K):
                        for ci in range(n_cin):
                            t = wp.tile([cin_p, C], FP16, tag=f"w{li}{p}_{k}_{ci}",
                                        name=f"w{li}{p}_{k}_{ci}")
                            nc.sync.dma_start(
                                out=t,
                                in_=wts[li, p][k, ci * cin_p:(ci + 1) * cin_p, :])
                            wt[p, k, ci] = t
                return wt

            def emit_out(li, b, co, src_ap_writer, tw, out_lo, pad_lo, pad_hi,
                         out_kind, conv_out):
                """src_ap_writer(dest_ap): emits the ACT op(s) writing the
                activated output rows into dest_ap (width tw)."""
                w_out = pad_lo + tw + pad_hi
                if out_kind == "f":
                    ot = conv_out.tile([128, w_out], F32, tag="of", bufs=3,
                                       name=f"of{li}_{co}")
                    if pad_lo:
                        nc.vector.memset(ot[:, 0:pad_lo], 0.0)
                    if pad_hi:
                        nc.vector.memset(ot[:, pad_lo + tw:w_out], 0.0)
                    src_ap_writer(ot[:, pad_lo:pad_lo + tw])
                    nc.sync.dma_start(
                        out=act[li, b, co, "f"][:, out_lo:out_lo + w_out],
                        in_=ot[:, :])
                else:
                    y32 = conv_out.tile([128, tw], F32, tag="y", bufs=3,
                                        name=f"y{li}_{co}")
                    src_ap_writer(y32[:, :])
                    oh = None
                    for p in ("h", "l"):
                        ot = conv_out.tile([128, w_out], FP16, tag=f"o{p}",
                                           bufs=4, name=f"o{li}_{co}{p}")
                        if pad_lo:
                            nc.vector.memset(ot[:, 0:pad_lo], 0.0)
                        if pad_hi:
                            nc.vector.memset(ot[:, pad_lo + tw:w_out], 0.0)
                        if p == "h":
                            nc.vector.tensor_copy(ot[:, pad_lo:pad_lo + tw],
                                                  y32[:, :])
                            oh = ot
                        else:
                            nc.vector.tensor_sub(ot[:, pad_lo:pad_lo + tw],
                                                 y32[:, :],
                                                 oh[:, pad_lo:pad_lo + tw])
                        nc.sync.dma_start(
                            out=act[li, b, co, p][:, out_lo:out_lo + w_out],
                            in_=ot[:, :])

            def conv_layer(li, K, tiles_spec, get_in, wt, n_cin, psump,
                           conv_in, conv_out, out_kind, batches=None):
                cin_p = wt["h", 0, 0].shape[0]
                for b in (batches if batches is not None else range(B_LOC)):
                    for (t0, tw, in_lo, in_w, out_lo, pad_lo, pad_hi,
                         stride) in tiles_spec:
                        ins = {}
                        for ci in range(n_cin):
                            for p in ("h", "l"):
                                it = conv_in.tile([cin_p, in_w], FP16,
                                                  tag=f"in{ci}{p}", bufs=2,
                                                  name=f"cin{li}_{ci}{p}")
                                nc.sync.dma_start(
                                    out=it, in_=get_in(b, ci, p, in_lo, in_w))
                                ins[ci, p] = it
                        for co in range(NCH):
                            ps = psump.tile([128, tw], F32, tag="cps")
                            nmm = n_cin * K * 3
                            i = 0
                            for ci in range(n_cin):
                                for k in range(K):
                                    if stride == 1:
                                        sl = slice(k, k + tw)
                                    else:
                                        sl = slice(k, k + 2 * (tw - 1) + 1, 2)
                                    wh = wt["h", k, ci][:, co * 128:(co + 1) * 128]
                                    wl = wt["l", k, ci][:, co * 128:(co + 1) * 128]
                                    for lhsT, rhs in ((wh, ins[ci, "h"][:, sl]),
                                                      (wh, ins[ci, "l"][:, sl]),
                                                      (wl, ins[ci, "h"][:, sl])):
                                        nc.tensor.matmul(ps[:, :], lhsT, rhs,
                                                         start=(i == 0),
                                                         stop=(i == nmm - 1))
                                        i += 1

                            def write(dest, ps=ps, li=li, co=co):
                                nc.scalar.activation(
                                    out=dest, in_=ps[:, :], func=RELU,
                                    bias=bn_b[li - 1, co][:, :],
                                    scale=bn_s[li - 1, co][:, :])
                            emit_out(li, b, co, write, tw, out_lo, pad_lo,
                                     pad_hi, out_kind, conv_out)

            # D-transform index specs for F(2,3): (in0_off, in1_off, op)
            DSPEC = [(0, 2, "sub"), (1, 2, "add"), (2, 1, "sub"), (1, 3, "sub")]

            def conv_layer_wino(li, tiles_spec, get_in32, wt, psump,
                                conv_in, conv_out, out_kind):
                J = 256
                for b in range(B_LOC):
                    for (t0, tw, in_lo, in_w, out_lo, pad_lo, pad_hi) in tiles_spec:
                        dh = {}
                        dl = {}
                        for ci in range(NCH):
                            x32 = conv_in.tile([128, in_w], F32, tag=f"x{ci}",
                                               bufs=2, name=f"x{li}_{ci}")
                            nc.sync.dma_start(out=x32,
                                              in_=get_in32(b, ci, in_lo, in_w))
                            for i, (a0, a1, op) in enumerate(DSPEC):
                                s0 = x32[:, slice(a0, a0 + 2 * (J - 1) + 1, 2)]
                                s1 = x32[:, slice(a1, a1 + 2 * (J - 1) + 1, 2)]
                                d32 = conv_in.tile([128, J], F32, tag="d32",
                                                   bufs=4, name=f"d32_{li}")
                                if op == "sub":
                                    nc.vector.tensor_sub(d32[:, :], s0, s1)
                                else:
                                    nc.vector.tensor_add(d32[:, :], s0, s1)
                                h = conv_in.tile([128, J], FP16,
                                                 tag=f"dh{ci}_{i}", bufs=2,
                                                 name=f"dh{li}_{ci}_{i}")
                                nc.scalar.activation(out=h[:, :], in_=d32[:, :],
                                                     func=COPY)
                                dsub = conv_in.tile([128, J], F32, tag="dsub",
                                                    bufs=4, name=f"dsub_{li}")
                                nc.vector.tensor_sub(dsub[:, :], d32[:, :],
                                                     h[:, :])
                                l = conv_in.tile([128, J], FP16,
                                                 tag=f"dl{ci}_{i}", bufs=2,
                                                 name=f"dl{li}_{ci}_{i}")
                                nc.vector.tensor_copy(l[:, :], dsub[:, :])
                                dh[ci, i] = h
                                dl[ci, i] = l
                        for co in range(NCH):
                            Mp = psump.tile([128, 4, J], F32, tag="M")
                            for i in range(4):
                                t = 0
                                for ci in range(NCH):
                                    gh = wt["h", i, ci][:, co * 128:(co + 1) * 128]
                                    gl = wt["l", i, ci][:, co * 128:(co + 1) * 128]
                                    for lhsT, rhs in ((gh, dh[ci, i]),
                                                      (gh, dl[ci, i]),
                                                      (gl, dh[ci, i])):
                                        nc.tensor.matmul(
                                            Mp[:, i, :], lhsT, rhs[:, :],
                                            start=(t == 0),
                                            stop=(t == NCH * 3 - 1))
                                        t += 1
                            # assembly: ye = m0+m1+m2 ; yo = m1-m2-m3
                            m1s = conv_out.tile([128, J], F32, tag="m1s",
                                                bufs=2, name=f"m1s{li}")
                            nc.scalar.activation(out=m1s[:, :],
                                                 in_=Mp[:, 1, :], func=COPY)
                            m2s = conv_out.tile([128, J], F32, tag="m2s",
                                                bufs=2, name=f"m2s{li}")
                            nc.scalar.activation(out=m2s[:, :],
                                                 in_=Mp[:, 2, :], func=COPY)
                            ye1 = conv_out.tile([128, J], F32, tag="ye1",
                                                bufs=2, name=f"ye1{li}")
                            nc.vector.tensor_add(ye1[:, :], Mp[:, 0, :],
                                                 m1s[:, :])
                            ye2 = conv_out.tile([128, J], F32, tag="ye2",
                                                bufs=2, name=f"ye2{li}")
                            nc.vector.tensor_add(ye2[:, :], ye1[:, :],
                                                 m2s[:, :])
                            yo1 = conv_out.tile([128, J], F32, tag="yo1",
                                                bufs=2, name=f"yo1{li}")
                            nc.vector.tensor_sub(yo1[:, :], m1s[:, :],
                                                 m2s[:, :])
                            yo2 = conv_out.tile([128, J], F32, tag="yo2",
                                                bufs=2, name=f"yo2{li}")
                            nc.vector.tensor_sub(yo2[:, :], yo1[:, :],
                                                 Mp[:, 3, :])

                            def write(dest, li=li, co=co, ye2=ye2, yo2=yo2,
                                      tw=tw):
                                nc.scalar.activation(
                                    out=dest[:, slice(0, tw, 2)],
                                    in_=ye2[:, 0:(tw + 1) // 2], func=RELU,
                                    bias=bn_b[li - 1, co][:, :],
                                    scale=bn_s[li - 1, co][:, :])
                                nc.scalar.activation(
                                    out=dest[:, slice(1, tw, 2)],
                                    in_=yo2[:, 0:tw // 2], func=RELU,
                                    bias=bn_b[li - 1, co][:, :],
                                    scale=bn_s[li - 1, co][:, :])
                            emit_out(li, b, co, write, tw, out_lo, pad_lo,
                                     pad_hi, out_kind, conv_out)

            # ---- L1: direct conv(80->768, K=3, valid) -> act1 fp32 ---------
            with tc.tile_pool(name="w1p", bufs=1) as wp, \
                 tc.tile_pool(name="c1i", bufs=1) as conv_in, \
                 tc.tile_pool(name="c1o", bufs=1) as conv_out, \
                 tc.tile_pool(name="ps1", bufs=2, space="PSUM") as psump:
                wt = load_weights(wp, 1, 3, CIN, 1)
                emit_bn0_dmas()
                emit_const_dmas()
                spec = [(0, 511, 0, 513, 0, 1, 0, 1),
                        (511, 511, 511, 513, 512, 0, 3, 1)]
                mels_d = {"h": mels_h, "l": mels_l}
                conv_layer(1, 3, spec,
                           lambda b, ci, p, lo, w: mels_d[p][b, :, lo:lo + w],
                           wt, 1, psump, conv_in, conv_out, "f")

            # ---- L2: Winograd conv(768->768) -> act2 h/l -------------------
            with tc.tile_pool(name="w2p", bufs=1) as wp, \
                 tc.tile_pool(name="c2i", bufs=1) as conv_in, \
                 tc.tile_pool(name="c2o", bufs=1) as conv_out, \
                 tc.tile_pool(name="ps2", bufs=2, space="PSUM") as psump:
                wt = load_weights(wp, 2, 4, 128, NCH)
                spec = [(0, 511, 0, 514, 0, 1, 0), (511, 511, 511, 514, 512, 0, 1)]
                conv_layer_wino(2, spec,
                                lambda b, ci, lo, w: act[1, b, ci, "f"][:, lo:lo + w],
                                wt, psump, conv_in, conv_out, "hl")

            # ---- L3: direct strided conv -> act3 fp32 ----------------------
            # Split into two T-tiles (halves the input-tile SBUF) so L4's
            # full weight set prefetches during L3's tail; emitted after two
            # batches so the DMAs don't compete with L3's startup loads.
            with tc.tile_pool(name="w4p", bufs=1) as wp4:
                with tc.tile_pool(name="w3p", bufs=1) as wp, \
                     tc.tile_pool(name="c3i", bufs=1) as conv_in, \
                     tc.tile_pool(name="c3o", bufs=1) as conv_out, \
                     tc.tile_pool(name="ps3", bufs=2, space="PSUM") as psump:
                    wt = load_weights(wp, 3, 4, 128, NCH)
                    spec = [(0, 256, 0, 514, 0, 1, 0, 2),
                            (256, 255, 512, 512, 257, 0, 4, 2)]
                    gi3 = lambda b, ci, p, lo, w: act[2, b, ci, p][:, lo:lo + w]
                    conv_layer(3, 4, spec, gi3, wt, NCH, psump,
                               conv_in, conv_out, "f", batches=[0, 1])
                    wt4 = load_weights(wp4, 4, 4, 128, NCH)
                    conv_layer(3, 4, spec, gi3, wt, NCH, psump,
                               conv_in, conv_out, "f", batches=[2, 3])

                # ---- L4: Winograd -> act4 fp32 -----------------------------
                with tc.tile_pool(name="c4i", bufs=1) as conv_in, \
                     tc.tile_pool(name="c4o", bufs=1) as conv_out, \
                     tc.tile_pool(name="ps4", bufs=2, space="PSUM") as psump:
                    spec = [(0, 511, 0, 514, 0, 1, 4)]
                    conv_layer_wino(4, spec,
                                    lambda b, ci, lo, w: act[3, b, ci, "f"][:, lo:lo + w],
                                    wt4, psump, conv_in, conv_out, "f")

            # ---- L5: Winograd -> act5 h/l ----------------------------------
            with tc.tile_pool(name="w5p", bufs=1) as wp, \
                 tc.tile_pool(name="c5i", bufs=1) as conv_in, \
                 tc.tile_pool(name="c5o", bufs=1) as conv_out, \
                 tc.tile_pool(name="ps5", bufs=2, space="PSUM") as psump:
                wt = load_weights(wp, 5, 4, 128, NCH)
                spec = [(0, 511, 0, 514, 0, 0, 0)]
                conv_layer_wino(5, spec,
                                lambda b, ci, lo, w: act[4, b, ci, "f"][:, lo:lo + w],
                                wt, psump, conv_in, conv_out, "hl")

            # ---- L6 (1x1 conv + bias) and VQ ------------------------------
            with tc.tile_pool(name="w6p", bufs=1) as wp, \
                 tc.tile_pool(name="c6i", bufs=1) as conv_in, \
                 tc.tile_pool(name="vq", bufs=2) as vqp, \
                 tc.tile_pool(name="vqsmall", bufs=4) as vqsp, \
                 tc.tile_pool(name="ps6", bufs=2, space="PSUM") as psump:
                wt6 = {}
                for p in ("h", "l"):
                    for ci in range(NCH):
                        t = wp.tile([128, D], FP16, tag=f"w6{p}_{ci}",
                                    name=f"w6{p}_{ci}")
                        nc.sync.dma_start(
                            out=t, in_=wts[6, p][ci * 128:(ci + 1) * 128, :])
                        wt6[p, ci] = t
                for b in range(B_LOC):
                    ins = {}
                    for ci in range(NCH):
                        for p in ("h", "l"):
                            it = conv_in.tile([128, T2], FP16, tag=f"in{ci}{p}",
                                              bufs=2, name=f"cin6_{ci}{p}")
                            nc.sync.dma_start(out=it, in_=act[5, b, ci, p][:, :])
                            ins[ci, p] = it
                    ps = psump.tile([D, T2], F32, tag="zps")
                    nmm = NCH * 3
                    i = 0
                    for ci in range(NCH):
                        for lhsT, rhs in ((wt6["h", ci], ins[ci, "h"]),
                                          (wt6["h", ci], ins[ci, "l"]),
                                          (wt6["l", ci], ins[ci, "h"])):
                            nc.tensor.matmul(ps[:, :], lhsT[:, :], rhs[:, :],
                                             start=(i == 0), stop=(i == nmm - 1))
                            i += 1
                    zb = vqp.tile([D, T2], F32, tag="zb")
                    nc.vector.tensor_scalar_add(zb[:, :], ps[:, :], b6s[:, :])
                    nc.sync.dma_start(out=z_out[b, :, :], in_=zb[:, :])

                    zsq = vqp.tile([D, T2], F32, tag="zsq")
                    nc.vector.tensor_mul(zsq[:, :], zb[:, :], zb[:, :])
                    for c in range(4):
                        c0 = c * 128
                        cs = min(128, T2 - c0)
                        x2p = psump.tile([128, 1], F32, tag="x2p")
                        nc.tensor.matmul(x2p[:cs, :], zsq[:, c0:c0 + cs],
                                         ones64[:, :], start=True, stop=True)
                        x2s = vqsp.tile([128, 1], F32, tag="x2s")
                        nc.vector.tensor_copy(x2s[:cs, :], x2p[:cs, :])
                        scp = psump.tile([128, M], F32, tag="scp")
                        nc.tensor.matmul(scp[:cs, :], zb[:, c0:c0 + cs],
                                         embT2s[:, :], start=True, stop=True)
                        t1 = vqsp.tile([128, M], F32, tag="t1")
                        nc.vector.tensor_scalar_sub(t1[:cs, :], e2ns[:cs, :],
                                                    x2s[:cs, :])
                        nd = vqsp.tile([128, M], F32, tag="nd")
                        nc.vector.tensor_add(nd[:cs, :], t1[:cs, :], scp[:cs, :])
                        mx = vqsp.tile([128, 8], F32, tag="mx")
                        nc.vector.max(mx[:cs, :], nd[:cs, :])
                        mi = vqsp.tile([128, 8], U32, tag="mi")
                        nc.vector.max_index(mi[:cs, :], mx[:cs, :], nd[:cs, :])
                        col = b * 4 + c
                        nc.vector.tensor_copy(idxacc[:cs, col:col + 1],
                                              mi[:cs, 0:1])
            nc.sync.dma_start(out=idx_out[:, :], in_=idxacc[:, :])

    nc.compile()
    return nc


def _get_nc():
    global _CACHED_NC
    if _CACHED_NC is None:
        _CACHED_NC = _build_nc()
    return _CACHED_NC


def _split_hl(x):
    h = x.astype(np.float16)
    l = (x - h.astype(np.float32)).astype(np.float16)
    return np.ascontiguousarray(h), np.ascontiguousarray(l)


def _host_prep(inputs):
    f = np.float32
    out = {}
    for li, key in ((1, "w1"), (3, "w3")):
        wT = np.ascontiguousarray(inputs[key].astype(f).transpose(2, 1, 0))
        out[f"w{li}Th"], out[f"w{li}Tl"] = _split_hl(wT)
    for li, key in ((2, "w2"), (4, "w4"), (5, "w5")):
        wT = inputs[key].astype(f).transpose(2, 1, 0)   # [3, Cin, Cout]
        g = np.empty((4,) + wT.shape[1:], f)
        g[0] = wT[0]
        g[1] = (wT[0] + wT[1] + wT[2]) * f(0.5)
        g[2] = (wT[0] - wT[1] + wT[2]) * f(0.5)
        g[3] = wT[2]
        out[f"w{li}Gh"], out[f"w{li}Gl"] = _split_hl(g)
    w6 = np.ascontiguousarray(inputs["w6"].astype(f)[:, :, 0].T)
    out["w6Th"], out["w6Tl"] = _split_hl(w6)
    gamma = inputs["bn_gamma"].astype(f)
    beta = inputs["bn_beta"].astype(f)
    mean = inputs["bn_mean"].astype(f)
    var = inputs["bn_var"].astype(f)
    inv = gamma / np.sqrt(var + f(1e-5))
    bias = beta - mean * inv
    out["bnS"] = np.ascontiguousarray(inv.reshape(5, NCH, 128, 1))
    out["bnB"] = np.ascontiguousarray(bias.reshape(5, NCH, 128, 1))
    out["b6v"] = np.ascontiguousarray(inputs["b6"].astype(f).reshape(D, 1))
    emb = inputs["embedding"].astype(f)
    out["embT2"] = np.ascontiguousarray(2.0 * emb.T)
    e2 = np.sum(emb.astype(np.float64) ** 2, axis=1).astype(f)
    out["e2n"] = np.ascontiguousarray(np.broadcast_to(-e2[None, :], (128, M)))
    return out, emb


def _make_in_maps(inputs):
    shared, emb = _host_prep(inputs)
    mels = inputs["mels"].astype(np.float32)
    B = mels.shape[0]
    assert B == N_CORES * B_LOC
    in_maps = []
    for c in range(N_CORES):
        m = dict(shared)
        mh, ml = _split_hl(mels[c * B_LOC:(c + 1) * B_LOC])
        m["mels_h"] = mh
        m["mels_l"] = ml
        in_maps.append(m)
    return in_maps, emb


def kernel(**inputs):
    nc = _get_nc()
    in_maps, emb = _make_in_maps(inputs)
    B = N_CORES * B_LOC

    res = run_bass_kernel_spmd(nc, in_maps, core_ids=list(range(N_CORES)))

    z_parts = []
    idx_parts = []
    for c in range(N_CORES):
        r = res.results[c]
        z_parts.append(r["z_out"])
        arr = r["idx_out"]
        loc = np.empty(B_LOC * T2, dtype=np.int64)
        for b in range(B_LOC):
            for ch in range(4):
                c0 = ch * 128
                cs = min(128, T2 - c0)
                loc[b * T2 + c0: b * T2 + c0 + cs] = arr[:cs, b * 4 + ch]
        idx_parts.append(loc)

    z = np.concatenate(z_parts, axis=0).transpose(0, 2, 1)
    z = np.ascontiguousarray(z)
    idx = np.concatenate(idx_parts)

    q = emb[idx].reshape(B, T2, D)
    q_st = z + (q - z)
    diff = z.astype(np.float64) - q.astype(np.float64)
    loss = np.float32(0.25 * np.mean(diff * diff))
    counts = np.bincount(idx, minlength=M).astype(np.float64)
    avg = counts / idx.shape[0]
    perplexity = np.float32(np.exp(-np.sum(avg * np.log(avg + 1e-10))))
    return q_st, loss, perplexity


# revision 18
# speedup vs baseline: 1.0474x; 1.0040x over previous
"""Trainium2 Bass kernel for nn_Encoder (conv stack + VQ codebook).

Reference computation (fp32):
  x = mels [32, 80, 1024]
  5x (conv1d + batchnorm-affine + relu), 1x 1x1-conv + bias  -> z [32, 64, 511]
  VQ: nearest codebook row (squared L2, 512 codes, D=64) -> q_st, loss, perplexity

Sharding: data-parallel over batch across 8 NeuronCores (4 batches/core);
conv weights + codebook replicated.

Speed tricks (all verified to keep VQ argmins identical to the fp32 ref):
- fp16 hi/lo 3-term matmuls (W=Wh+Wl, X=Xh+Xl; Y ~= Wh.Xh + Wh.Xl + Wl.Xh):
  1 PE cycle/row instead of fp32's 4, output rel err ~3e-6 (the PE supports
  fp16 subnormals, so the lo parts need no scaling).
- Winograd F(2,3) on the stride-1 K=3 768x768 layers (L2/L4/L5): 4 transformed
  256-col matmuls per output pair instead of 3x511 direct columns (1.5x fewer
  PE cycles), transforms on DVE/ACT in fp32.
- The VQ distance computation stays fully fp32 and mirrors the reference's
  operation order; host finishes gather/loss/perplexity exactly as the
  reference does.
"""

import numpy as np

import concourse.bass as bass
import concourse.tile as tile
from concourse import bacc, mybir
from concourse.bass_utils import run_bass_kernel_spmd

N_CORES = 8
B_LOC = 4
CIN = 80
C = 768
D = 64
M = 512
T0 = 1024
T2 = 511
NCH = C // 128
F32 = mybir.dt.float32
FP16 = mybir.dt.float16
U32 = mybir.dt.uint32
RELU = mybir.ActivationFunctionType.Relu
COPY = mybir.ActivationFunctionType.Copy

_CACHED_NC = None


def _build_nc():
    nc = bacc.Bacc("TRN2", target_bir_lowering=False, debug=False,
                   num_devices=N_CORES)

    mels_h = nc.dram_tensor("mels_h", [B_LOC, CIN, T0], FP16, kind="ExternalInput")
    mels_l = nc.dram_tensor("mels_l", [B_LOC, CIN, T0], FP16, kind="ExternalInput")
    wts = {}
    for li, K, cin in ((1, 3, CIN), (3, 4, C)):
        for p in ("h", "l"):
            wts[li, p] = nc.dram_tensor(f"w{li}T{p}", [K, cin, C], FP16,
                                        kind="ExternalInput")
    for li in (2, 4, 5):  # Winograd G-transformed weights
        for p in ("h", "l"):
            wts[li, p] = nc.dram_tensor(f"w{li}G{p}", [4, C, C], FP16,
                                        kind="ExternalInput")
    for p in ("h", "l"):
        wts[6, p] = nc.dram_tensor(f"w6T{p}", [C, D], FP16, kind="ExternalInput")
    bnS = nc.dram_tensor("bnS", [5, NCH, 128, 1], F32, kind="ExternalInput")
    bnB = nc.dram_tensor("bnB", [5, NCH, 128, 1], F32, kind="ExternalInput")
    b6v = nc.dram_tensor("b6v", [D, 1], F32, kind="ExternalInput")
    embT2 = nc.dram_tensor("embT2", [D, M], F32, kind="ExternalInput")
    e2n = nc.dram_tensor("e2n", [128, M], F32, kind="ExternalInput")

    z_out = nc.dram_tensor("z_out", [B_LOC, D, T2], F32, kind="ExternalOutput")
    idx_out = nc.dram_tensor("idx_out", [128, 16], U32, kind="ExternalOutput")

    with tile.TileContext(nc) as tc:
        with (
            tc.tile_pool(name="const", bufs=1) as constp,
            tc.tile_pool(name="dram", bufs=1, space="DRAM") as dramp,
        ):
            # Const tiles are allocated here, but their (slow, strided) DMAs
            # are emitted after L1's weight loads, on the gpsimd SWDGE queues,
            # so they don't delay the first conv matmuls.
            bn_s = {}
            bn_b = {}
            for li in range(5):
                for ci in range(NCH):
                    s = constp.tile([128, 1], F32, tag=f"bns_{li}_{ci}")
                    b = constp.tile([128, 1], F32, tag=f"bnb_{li}_{ci}")
                    bn_s[li, ci] = s
                    bn_b[li, ci] = b
            b6s = constp.tile([D, 1], F32, tag="b6s")
            embT2s = constp.tile([D, M], F32, tag="embT2s")
            e2ns = constp.tile([128, M], F32, tag="e2ns")
            ones64 = constp.tile([D, 1], F32, tag="ones64")
            nc.vector.memset(ones64, 1.0)
            idxacc = constp.tile([128, 16], U32, tag="idxacc")

            def emit_bn0_dmas():
                # L1's ReLU scale/bias must arrive fast or the first psum
                # drains block and the PE never warms (HAM stays at K=4/8).
                for ci in range(NCH):
                    nc.sync.dma_start(out=bn_s[0, ci], in_=bnS[0, ci, :, :])
                    nc.sync.dma_start(out=bn_b[0, ci], in_=bnB[0, ci, :, :])

            def emit_const_dmas():
                for li in range(1, 5):
                    for ci in range(NCH):
                        nc.gpsimd.dma_start(out=bn_s[li, ci],
                                            in_=bnS[li, ci, :, :])
                        nc.gpsimd.dma_start(out=bn_b[li, ci],
                                            in_=bnB[li, ci, :, :])
                nc.gpsimd.dma_start(out=b6s, in_=b6v[:, :])
                nc.gpsimd.dma_start(out=embT2s, in_=embT2[:, :])
                nc.gpsimd.dma_start(out=e2ns, in_=e2n[:, :])

            # DRAM scratch. fp32 acts feed Winograd layers; fp16 h/l pairs
            # feed the direct layers (L3, L6).
            act = {}
            for li, width, kind in ((1, 1026, "f"), (2, T0, "hl"),
                                    (3, 516, "f"), (4, 516, "f"),
                                    (5, T2, "hl")):
                for b in range(B_LOC):
                    for ci in range(NCH):
                        if kind == "f":
                            act[li, b, ci, "f"] = dramp.tile(
                                [128, width], F32, tag=f"a{li}_{b}_{ci}",
                                name=f"a{li}_{b}_{ci}")
                        else:
                            for p in ("h", "l"):
                                act[li, b, ci, p] = dramp.tile(
                                    [128, width], FP16, tag=f"a{li}_{b}_{ci}{p}",
                                    name=f"a{li}_{b}_{ci}{p}")

            def load_weights(wp, li, K, cin_p, n_cin):
                wt = {}
                for p in ("h", "l"):
                    for k in range(K):
                        for ci in range(n_cin):
                            t = wp.tile([cin_p, C], FP16, tag=f"w{li}{p}_{k}_{ci}",
                                        name=f"w{li}{p}_{k}_{ci}")
                            nc.sync.dma_start(
                                out=t,
                                in_=wts[li, p][k, ci * cin_p:(ci + 1) * cin_p, :])
                            wt[p, k, ci] = t
                return wt

            def emit_out(li, b, co, src_ap_writer, tw, out_lo, pad_lo, pad_hi,
                         out_kind, conv_out):
                """src_ap_writer(dest_ap): emits the ACT op(s) writing the
                activated output rows into dest_ap (width tw)."""
                w_out = pad_lo + tw + pad_hi
                if out_kind == "f":
                    ot = conv_out.tile([128, w_out], F32, tag="of", bufs=3,
                                       name=f"of{li}_{co}")
                    if pad_lo:
                        nc.vector.memset(ot[:, 0:pad_lo], 0.0)
                    if pad_hi:
                        nc.vector.memset(ot[:, pad_lo + tw:w_out], 0.0)
                    src_ap_writer(ot[:, pad_lo:pad_lo + tw])
                    nc.sync.dma_start(
                        out=act[li, b, co, "f"][:, out_lo:out_lo + w_out],
                        in_=ot[:, :])
                else:
                    y32 = conv_out.tile([128, tw], F32, tag="y", bufs=3,
                                        name=f"y{li}_{co}")
                    src_ap_writer(y32[:, :])
                    oh = None
                    for p in ("h", "l"):
                        ot = conv_out.tile([128, w_out], FP16, tag=f"o{p}",
                                           bufs=4, name=f"o{li}_{co}{p}")
                        if pad_lo:
                            nc.vector.memset(ot[:, 0:pad_lo], 0.0)
                        if pad_hi:
                            nc.vector.memset(ot[:, pad_lo + tw:w_out], 0.0)
                        if p == "h":
                            nc.vector.tensor_copy(ot[:, pad_lo:pad_lo + tw],
                                                  y32[:, :])
                            oh = ot
                        else:
                            nc.vector.tensor_sub(ot[:, pad_lo:pad_lo + tw],
                                                 y32[:, :],
                                                 oh[:, pad_lo:pad_lo + tw])
                        nc.sync.dma_start(
                            out=act[li, b, co, p][:, out_lo:out_lo + w_out],
                            in_=ot[:, :])

            def conv_layer(li, K, tiles_spec, get_in, wt, n_cin, psump,
                           conv_in, conv_out, out_kind, batches=None):
                cin_p = wt["h", 0, 0].shape[0]
                for b in (batches if batches is not None else range(B_LOC)):
                    for (t0, tw, in_lo, in_w, out_lo, pad_lo, pad_hi,
                         stride) in tiles_spec:
                        ins = {}
                        for ci in range(n_cin):
                            for p in ("h", "l"):
                                it = conv_in.tile([cin_p, in_w], FP16,
                                                  tag=f"in{ci}{p}", bufs=2,
                                                  name=f"cin{li}_{ci}{p}")
                                nc.sync.dma_start(
                                    out=it, in_=get_in(b, ci, p, in_lo, in_w))
                                ins[ci, p] = it
                        for co in range(NCH):
                            ps = psump.tile([128, tw], F32, tag="cps")
                            nmm = n_cin * K * 3
                            i = 0
                            for ci in range(n_cin):
                                for k in range(K):
                                    if stride == 1:
                                        sl = slice(k, k + tw)
                                    else:
                                        sl = slice(k, k + 2 * (tw - 1) + 1, 2)
                                    wh = wt["h", k, ci][:, co * 128:(co + 1) * 128]
                                    wl = wt["l", k, ci][:, co * 128:(co + 1) * 128]
                                    for lhsT, rhs in ((wh, ins[ci, "h"][:, sl]),
                                                      (wh, ins[ci, "l"][:, sl]),
                                                      (wl, ins[ci, "h"][:, sl])):
                                        nc.tensor.matmul(ps[:, :], lhsT, rhs,
                                                         start=(i == 0),
                                                         stop=(i == nmm - 1))
                                        i += 1

                            def write(dest, ps=ps, li=li, co=co):
                                nc.scalar.activation(
                                    out=dest, in_=ps[:, :], func=RELU,
                                    bias=bn_b[li - 1, co][:, :],
                                    scale=bn_s[li - 1, co][:, :])
                            emit_out(li, b, co, write, tw, out_lo, pad_lo,
                                     pad_hi, out_kind, conv_out)

            # D-transform index specs for F(2,3): (in0_off, in1_off, op)
            DSPEC = [(0, 2, "sub"), (1, 2, "add"), (2, 1, "sub"), (1, 3, "sub")]

            def conv_layer_wino(li, tiles_spec, get_in32, wt, psump,
                                conv_in, conv_out, out_kind):
                J = 256
                for b in range(B_LOC):
                    for (t0, tw, in_lo, in_w, out_lo, pad_lo, pad_hi) in tiles_spec:
                        dh = {}
                        dl = {}
                        for ci in range(NCH):
                            x32 = conv_in.tile([128, in_w], F32, tag=f"x{ci}",
                                               bufs=2, name=f"x{li}_{ci}")
                            nc.sync.dma_start(out=x32,
                                              in_=get_in32(b, ci, in_lo, in_w))
                            for i, (a0, a1, op) in enumerate(DSPEC):
                                s0 = x32[:, slice(a0, a0 + 2 * (J - 1) + 1, 2)]
                                s1 = x32[:, slice(a1, a1 + 2 * (J - 1) + 1, 2)]
                                d32 = conv_in.tile([128, J], F32, tag="d32",
                                                   bufs=4, name=f"d32_{li}")
                                if op == "sub":
                                    nc.vector.tensor_sub(d32[:, :], s0, s1)
                                else:
                                    nc.vector.tensor_add(d32[:, :], s0, s1)
                                h = conv_in.tile([128, J], FP16,
                                                 tag=f"dh{ci}_{i}", bufs=2,
                                                 name=f"dh{li}_{ci}_{i}")
                                nc.scalar.activation(out=h[:, :], in_=d32[:, :],
                                                     func=COPY)
                                dsub = conv_in.tile([128, J], F32, tag="dsub",
                                                    bufs=4, name=f"dsub_{li}")
                                nc.vector.tensor_sub(dsub[:, :], d32[:, :],
                                                     h[:, :])
                                l = conv_in.tile([128, J], FP16,
                                                 tag=f"dl{ci}_{i}", bufs=2,
                                                 name=f"dl{li}_{ci}_{i}")
                                nc.vector.tensor_copy(l[:, :], dsub[:, :])
                                dh[ci, i] = h
                                dl[ci, i] = l
                        for co in range(NCH):
                            Mp = psump.tile([128, 4, J], F32, tag="M")
                            for i in range(4):
                                t = 0
                                for ci in range(NCH):
                                    gh = wt["h", i, ci][:, co * 128:(co + 1) * 128]
                                    gl = wt["l", i, ci][:, co * 128:(co + 1) * 128]
                                    for lhsT, rhs in ((gh, dh[ci, i]),
                                                      (gh, dl[ci, i]),
                                                      (gl, dh[ci, i])):
                                        nc.tensor.matmul(
                                            Mp[:, i, :], lhsT, rhs[:, :],
                                            start=(t == 0),
                                            stop=(t == NCH * 3 - 1))
                                        t += 1
                            # assembly: ye = m0+m1+m2 ; yo = m1-m2-m3
                            m1s = conv_out.tile([128, J], F32, tag="m1s",
                                                bufs=2, name=f"m1s{li}")
                            nc.scalar.activation(out=m1s[:, :],
                                                 in_=Mp[:, 1, :], func=COPY)
                            m2s = conv_out.tile([128, J], F32, tag="m2s",
                                                bufs=2, name=f"m2s{li}")
                            nc.scalar.activation(out=m2s[:, :],
                                                 in_=Mp[:, 2, :], func=COPY)
                            ye1 = conv_out.tile([128, J], F32, tag="ye1",
                                                bufs=2, name=f"ye1{li}")
                            nc.vector.tensor_add(ye1[:, :], Mp[:, 0, :],
                                                 m1s[:, :])
                            ye2 = conv_out.tile([128, J], F32, tag="ye2",
                                                bufs=2, name=f"ye2{li}")
                            nc.vector.tensor_add(ye2[:, :], ye1[:, :],
                                                 m2s[:, :])
                            yo1 = conv_out.tile([128, J], F32, tag="yo1",
                                                bufs=2, name=f"yo1{li}")
                            nc.vector.tensor_sub(yo1[:, :], m1s[:, :],
                                                 m2s[:, :])
                            yo2 = conv_out.tile([128, J], F32, tag="yo2",
                                                bufs=2, name=f"yo2{li}")
                            nc.vector.tensor_sub(yo2[:, :], yo1[:, :],
                                                 Mp[:, 3, :])

                            def write(dest, li=li, co=co, ye2=ye2, yo2=yo2,
                                      tw=tw):
                                nc.scalar.activation(
                                    out=dest[:, slice(0, tw, 2)],
                                    in_=ye2[:, 0:(tw + 1) // 2], func=RELU,
                                    bias=bn_b[li - 1, co][:, :],
                                    scale=bn_s[li - 1, co][:, :])
                                nc.scalar.activation(
                                    out=dest[:, slice(1, tw, 2)],
                                    in_=yo2[:, 0:tw // 2], func=RELU,
                                    bias=bn_b[li - 1, co][:, :],
                                    scale=bn_s[li - 1, co][:, :])
                            emit_out(li, b, co, write, tw, out_lo, pad_lo,
                                     pad_hi, out_kind, conv_out)

            # ---- L1: direct conv(80->768, K=3, valid) -> act1 fp32 ---------
            with tc.tile_pool(name="w1p", bufs=1) as wp, \
                 tc.tile_pool(name="c1i", bufs=1) as conv_in, \
                 tc.tile_pool(name="c1o", bufs=1) as conv_out, \
                 tc.tile_pool(name="ps1", bufs=2, space="PSUM") as psump:
                wt = load_weights(wp, 1, 3, CIN, 1)
                emit_bn0_dmas()
                emit_const_dmas()
                spec = [(0, 511, 0, 513, 0, 1, 0, 1),
                        (511, 511, 511, 513, 512, 0, 3, 1)]
                mels_d = {"h": mels_h, "l": mels_l}
                conv_layer(1, 3, spec,
                           lambda b, ci, p, lo, w: mels_d[p][b, :, lo:lo + w],
                           wt, 1, psump, conv_in, conv_out, "f")

            # ---- L2: Winograd conv(768->768) -> act2 h/l -------------------
            with tc.tile_pool(name="w2p", bufs=1) as wp, \
                 tc.tile_pool(name="c2i", bufs=1) as conv_in, \
                 tc.tile_pool(name="c2o", bufs=1) as conv_out, \
                 tc.tile_pool(name="ps2", bufs=3, space="PSUM") as psump:
                wt = load_weights(wp, 2, 4, 128, NCH)
                spec = [(0, 511, 0, 514, 0, 1, 0), (511, 511, 511, 514, 512, 0, 1)]
                conv_layer_wino(2, spec,
                                lambda b, ci, lo, w: act[1, b, ci, "f"][:, lo:lo + w],
                                wt, psump, conv_in, conv_out, "hl")

            # ---- L3: direct strided conv -> act3 fp32 ----------------------
            # Split into two T-tiles (halves the input-tile SBUF) so L4's
            # full weight set prefetches during L3's tail; emitted after two
            # batches so the DMAs don't compete with L3's startup loads.
            with tc.tile_pool(name="w4p", bufs=1) as wp4:
                with tc.tile_pool(name="w3p", bufs=1) as wp, \
                     tc.tile_pool(name="c3i", bufs=1) as conv_in, \
                     tc.tile_pool(name="c3o", bufs=1) as conv_out, \
                     tc.tile_pool(name="ps3", bufs=2, space="PSUM") as psump:
                    wt = load_weights(wp, 3, 4, 128, NCH)
                    spec = [(0, 256, 0, 514, 0, 1, 0, 2),
                            (256, 255, 512, 512, 257, 0, 4, 2)]
                    gi3 = lambda b, ci, p, lo, w: act[2, b, ci, p][:, lo:lo + w]
                    conv_layer(3, 4, spec, gi3, wt, NCH, psump,
                               conv_in, conv_out, "f", batches=[0, 1])
                    wt4 = load_weights(wp4, 4, 4, 128, NCH)
                    conv_layer(3, 4, spec, gi3, wt, NCH, psump,
                               conv_in, conv_out, "f", batches=[2, 3])

                # ---- L4: Winograd -> act4 fp32 -----------------------------
                with tc.tile_pool(name="c4i", bufs=1) as conv_in, \
                     tc.tile_pool(name="c4o", bufs=1) as conv_out, \
                     tc.tile_pool(name="ps4", bufs=3, space="PSUM") as psump:
                    spec = [(0, 511, 0, 514, 0, 1, 4)]
                    conv_layer_wino(4, spec,
                                    lambda b, ci, lo, w: act[3, b, ci, "f"][:, lo:lo + w],
                                    wt4, psump, conv_in, conv_out, "f")

            # ---- L5: Winograd -> act5 h/l ----------------------------------
            with tc.tile_pool(name="w5p", bufs=1) as wp, \
                 tc.tile_pool(name="c5i", bufs=1) as conv_in, \
                 tc.tile_pool(name="c5o", bufs=1) as conv_out, \
                 tc.tile_pool(name="ps5", bufs=3, space="PSUM") as psump:
                wt = load_weights(wp, 5, 4, 128, NCH)
                spec = [(0, 511, 0, 514, 0, 0, 0)]
                conv_layer_wino(5, spec,
                                lambda b, ci, lo, w: act[4, b, ci, "f"][:, lo:lo + w],
                                wt, psump, conv_in, conv_out, "hl")

            # ---- L6 (1x1 conv + bias) and VQ ------------------------------
            with tc.tile_pool(name="w6p", bufs=1) as wp, \
                 tc.tile_pool(name="c6i", bufs=1) as conv_in, \
                 tc.tile_pool(name="vq", bufs=2) as vqp, \
                 tc.tile_pool(name="vqsmall", bufs=4) as vqsp, \
                 tc.tile_pool(name="ps6", bufs=2, space="PSUM") as psump:
                wt6 = {}
                for p in ("h", "l"):
                    for ci in range(NCH):
                        t = wp.tile([128, D], FP16, tag=f"w6{p}_{ci}",
                                    name=f"w6{p}_{ci}")
                        nc.sync.dma_start(
                            out=t, in_=wts[6, p][ci * 128:(ci + 1) * 128, :])
                        wt6[p, ci] = t
                for b in range(B_LOC):
                    ins = {}
                    for ci in range(NCH):
                        for p in ("h", "l"):
                            it = conv_in.tile([128, T2], FP16, tag=f"in{ci}{p}",
                                              bufs=2, name=f"cin6_{ci}{p}")
                            nc.sync.dma_start(out=it, in_=act[5, b, ci, p][:, :])
                            ins[ci, p] = it
                    ps = psump.tile([D, T2], F32, tag="zps")
                    nmm = NCH * 3
                    i = 0
                    for ci in range(NCH):
                        for lhsT, rhs in ((wt6["h", ci], ins[ci, "h"]),
                                          (wt6["h", ci], ins[ci, "l"]),
                                          (wt6["l", ci], ins[ci, "h"])):
                            nc.tensor.matmul(ps[:, :], lhsT[:, :], rhs[:, :],
                                             start=(i == 0), stop=(i == nmm - 1))
                            i += 1
                    zb = vqp.tile([D, T2], F32, tag="zb")
                    nc.vector.tensor_scalar_add(zb[:, :], ps[:, :], b6s[:, :])
                    nc.sync.dma_start(out=z_out[b, :, :], in_=zb[:, :])

                    zsq = vqp.tile([D, T2], F32, tag="zsq")
                    nc.vector.tensor_mul(zsq[:, :], zb[:, :], zb[:, :])
                    for c in range(4):
                        c0 = c * 128
                        cs = min(128, T2 - c0)
                        x2p = psump.tile([128, 1], F32, tag="x2p")
                        nc.tensor.matmul(x2p[:cs, :], zsq[:, c0:c0 + cs],
                                         ones64[:, :], start=True, stop=True)
                        x2s = vqsp.tile([128, 1], F32, tag="x2s")
                        nc.vector.tensor_copy(x2s[:cs, :], x2p[:cs, :])
                        scp = psump.tile([128, M], F32, tag="scp")
                        nc.tensor.matmul(scp[:cs, :], zb[:, c0:c0 + cs],
                                         embT2s[:, :], start=True, stop=True)
                        t1 = vqsp.tile([128, M], F32, tag="t1")
                        nc.vector.tensor_scalar_sub(t1[:cs, :], e2ns[:cs, :],
                                                    x2s[:cs, :])
                        nd = vqsp.tile([128, M], F32, tag="nd")
                        nc.vector.tensor_add(nd[:cs, :], t1[:cs, :], scp[:cs, :])
                        mx = vqsp.tile([128, 8], F32, tag="mx")
                        nc.vector.max(mx[:cs, :], nd[:cs, :])
                        mi = vqsp.tile([128, 8], U32, tag="mi")
                        nc.vector.max_index(mi[:cs, :], mx[:cs, :], nd[:cs, :])
                        col = b * 4 + c
                        nc.vector.tensor_copy(idxacc[:cs, col:col + 1],
                                              mi[:cs, 0:1])
            nc.sync.dma_start(out=idx_out[:, :], in_=idxacc[:, :])

    nc.compile()
    return nc


def _get_nc():
    global _CACHED_NC
    if _CACHED_NC is None:
        _CACHED_NC = _build_nc()
    return _CACHED_NC


def _split_hl(x):
    h = x.astype(np.float16)
    l = (x - h.astype(np.float32)).astype(np.float16)
    return np.ascontiguousarray(h), np.ascontiguousarray(l)


def _host_prep(inputs):
    f = np.float32
    out = {}
    for li, key in ((1, "w1"), (3, "w3")):
        wT = np.ascontiguousarray(inputs[key].astype(f).transpose(2, 1, 0))
        out[f"w{li}Th"], out[f"w{li}Tl"] = _split_hl(wT)
    for li, key in ((2, "w2"), (4, "w4"), (5, "w5")):
        wT = inputs[key].astype(f).transpose(2, 1, 0)   # [3, Cin, Cout]
        g = np.empty((4,) + wT.shape[1:], f)
        g[0] = wT[0]
        g[1] = (wT[0] + wT[1] + wT[2]) * f(0.5)
        g[2] = (wT[0] - wT[1] + wT[2]) * f(0.5)
        g[3] = wT[2]
        out[f"w{li}Gh"], out[f"w{li}Gl"] = _split_hl(g)
    w6 = np.ascontiguousarray(inputs["w6"].astype(f)[:, :, 0].T)
    out["w6Th"], out["w6Tl"] = _split_hl(w6)
    gamma = inputs["bn_gamma"].astype(f)
    beta = inputs["bn_beta"].astype(f)
    mean = inputs["bn_mean"].astype(f)
    var = inputs["bn_var"].astype(f)
    inv = gamma / np.sqrt(var + f(1e-5))
    bias = beta - mean * inv
    out["bnS"] = np.ascontiguousarray(inv.reshape(5, NCH, 128, 1))
    out["bnB"] = np.ascontiguousarray(bias.reshape(5, NCH, 128, 1))
    out["b6v"] = np.ascontiguousarray(inputs["b6"].astype(f).reshape(D, 1))
    emb = inputs["embedding"].astype(f)
    out["embT2"] = np.ascontiguousarray(2.0 * emb.T)
    e2 = np.sum(emb.astype(np.float64) ** 2, axis=1).astype(f)
    out["e2n"] = np.ascontiguousarray(np.broadcast_to(-e2[None, :], (128, M)))
    return out, emb


def _make_in_maps(inputs):
    shared, emb = _host_prep(inputs)
    mels = inputs["mels"].astype(np.float32)
    B = mels.shape[0]
    assert B == N_CORES * B_LOC
    in_maps = []
    for c in range(N_CORES):
        m = dict(shared)
        mh, ml = _split_hl(mels[c * B_LOC:(c + 1) * B_LOC])
        m["mels_h"] = mh
        m["mels_l"] = ml
        in_maps.append(m)
    return in_maps, emb


def kernel(**inputs):
    nc = _get_nc()
    in_maps, emb = _make_in_maps(inputs)
    B = N_CORES * B_LOC

    res = run_bass_kernel_spmd(nc, in_maps, core_ids=list(range(N_CORES)))

    z_parts = []
    idx_parts = []
    for c in range(N_CORES):
        r = res.results[c]
        z_parts.append(r["z_out"])
        arr = r["idx_out"]
        loc = np.empty(B_LOC * T2, dtype=np.int64)
        for b in range(B_LOC):
            for ch in range(4):
                c0 = ch * 128
                cs = min(128, T2 - c0)
                loc[b * T2 + c0: b * T2 + c0 + cs] = arr[:cs, b * 4 + ch]
        idx_parts.append(loc)

    z = np.concatenate(z_parts, axis=0).transpose(0, 2, 1)
    z = np.ascontiguousarray(z)
    idx = np.concatenate(idx_parts)

    q = emb[idx].reshape(B, T2, D)
    q_st = z + (q - z)
    diff = z.astype(np.float64) - q.astype(np.float64)
    loss = np.float32(0.25 * np.mean(diff * diff))
    counts = np.bincount(idx, minlength=M).astype(np.float64)
    avg = counts / idx.shape[0]
    perplexity = np.float32(np.exp(-np.sum(avg * np.log(avg + 1e-10))))
    return q_st, loss, perplexity
